# revision 1
# baseline (speedup 1.0000x reference)
"""Trainium2 Bass kernel for additive (Bahdanau) attention.

  context[b] = sum_t softmax_t( v . tanh(We @ enc[b,t] + Wd @ dec[b] + bias) ) * enc[b,t]

Shapes (hardcoded): enc_out [64, 2048, 1024] f32, dec_state [64, 1024] f32,
W_weight [1024, 2048], W_bias [1024], v_weight [1, 1024].  Output [64, 1024].

Sharding: data-parallel over batch across 8 NeuronCores (8 batches/core).

Design (fp8 DoubleRow; ~334us/core vs 670us f32r baseline, rel err ~1.0e-2):
- Host prep: enc is cast twice -- bf16 in [b, tl, i, e] layout (the ctx
  matmul's moving stream) and fp8-e4m3 in pre-transposed [b, el, i, j, tl]
  layout (the proj matmul's stationary operand) -- so there are no on-device
  transposes at all.  We^T is scaled x32 and cast to fp8 in [el, j, d]
  pair layout (the scale keeps the small We values out of e4m3's subnormal
  range, where quantization error would dominate; tanh's scale=1/32 undoes
  it exactly).  z = 32*(Wd @ dec + bias) is bf16, replicated to 128
  partitions.
- proj = X @ (32 We^T): fp8 MatmulPerfMode.DoubleRow, 2 K-tiles per
  instruction.  On HW an N=512 matmul costs ~216ns regardless of K or
  dtype, so DR's win is halving the pass count: 8 matmuls/tile instead of
  16.  PSUM is allocated as per-half [128,512] tiles on a 5-deep ring.
- Epilogue per 128-row tile, spread over three engines so none exceeds the
  PE's ~2.4us/tile: DVE adds z (PSUM f32 + bf16 -> bf16, per half); ACT
  tanh(scale=1/32); DVE scalar_tensor_tensor (x v, accumulate) on cols
  [0:384); Pool tensor_tensor mult on [384:1024) with ACT Copy+accum_out
  reducing it one step later; ACT exp(s0 + bias=s1) two steps later (the
  deferrals keep the in-order ACT/DVE queues from ping-ponging).  Softmax
  needs no max-subtraction (|scores| <= sum|v| <= 32).
- ctx += p^T @ X accumulates in PSUM over a batch's 16 tiles (2 bf16
  matmuls/tile); per batch: DVE reduce (bf16) + one bf16 N=1 PE matmul give l = sum p,
  DVE reciprocal + ACT scaled copies normalize, DMA out.
- One global software pipeline over 128 row-tiles.  DMA is batched 4
  tiles per instruction (all runs >= 1KB contiguous) on the sync queue.
  Scheduling invariants learned from traces: prefetch depth must stay
  under each ring's buffer count (a DMA that waits for its ring buffer
  blocks the whole in-order sync queue); per-batch z tiles avoid WAR
  coupling; ctx matmuls must stay interleaved per-tile (bunching them
  serializes their PSUM accumulate chains).
"""
import sys

sys.path.insert(0, "/opt/trn_rl_repo")

from contextlib import ExitStack

import ml_dtypes
import numpy as np

import concourse.tile as tile
from concourse import bacc, mybir
from concourse.bass_utils import run_bass_kernel_spmd

F32 = mybir.dt.float32
BF16 = mybir.dt.bfloat16
FP8 = mybir.dt.float8e4
NP_FP8 = ml_dtypes.float8_e4m3
NP_BF16 = ml_dtypes.bfloat16
DR = mybir.MatmulPerfMode.DoubleRow

B, T, E, D = 64, 2048, 1024, 1024
CORES = 8
BL = B // CORES           # batches per core (8)
P = 128                   # partitions
TT = T // P               # t-tiles per batch (16)
ET = E // P               # e-blocks per row-tile (8)
QUAD = 4                  # t-tiles fetched per DMA instruction
CTX_LAG = 4               # t-tiles of lag before emitting ctx matmuls
EXP_LAG = 3               # t-tiles of lag before emitting exp (vs proj)
END_LAG = 2               # extra t-tiles before emitting batch-end chain
PREFETCH_QUADS = 4
WSCALE = 32.0             # fp8 subnormal-avoidance scale on We^T and z


def _build_kernel(bl=BL, t_tiles=TT):
    nc = bacc.Bacc(
        "TRN2",
        target_bir_lowering=False,
        debug=False,
        num_devices=CORES,
    )

    # [b, tl, i, e]: x16[b, tl, i, :] = enc[b, i*128+tl, :] in bf16
    x16 = nc.declare_dram_parameter("x16", [bl, P, t_tiles, E], BF16, isOutput=False)
    # [b, el, i, j, tl]: xt8[b, el, i, j, tl] = enc[b, i*128+tl, j*128+el] in fp8
    xt8 = nc.declare_dram_parameter("xt8", [bl, P, t_tiles, ET, P], FP8, isOutput=False)
    # [el, j, d]: wet8[el, j, d] = 32 * We[d, j*128+el] in fp8
    wet8 = nc.declare_dram_parameter("wet8", [P, ET, D], FP8, isOutput=False)
    # [k, b, d]: 32 * z[b, d] in bf16, replicated over k partitions
    zrep16 = nc.declare_dram_parameter("zrep16", [P, bl, D], BF16, isOutput=False)
    v16 = nc.declare_dram_parameter("v16", [P, D], BF16, isOutput=False)
    onesc = nc.declare_dram_parameter("onesc", [P, 1], BF16, isOutput=False)
    out = nc.declare_dram_parameter("ctx_out", [bl, E], F32, isOutput=True)

    n_quads_total = bl * t_tiles // QUAD

    with tile.TileContext(nc) as tc, ExitStack() as ctx:
        const = ctx.enter_context(tc.tile_pool(name="const", bufs=1))
        xq_pool = ctx.enter_context(tc.tile_pool(name="xq", bufs=6))
        xtq_pool = ctx.enter_context(tc.tile_pool(name="xtq", bufs=6))
        epool = ctx.enter_context(tc.tile_pool(name="e", bufs=4))
        small = ctx.enter_context(tc.tile_pool(name="small", bufs=2))

        ps_proj = ctx.enter_context(tc.tile_pool(name="ps_proj", bufs=5, space="PSUM"))
        ps_ctx = ctx.enter_context(tc.tile_pool(name="ps_ctx", bufs=2, space="PSUM"))
        ps_misc = ctx.enter_context(tc.tile_pool(name="ps_misc", bufs=1, space="PSUM"))

        # ---- resident constants.  Ordered so proj(0)'s inputs land first.
        xq_tiles = {}
        xtq_tiles = {}

        def fetch_quad(q, skip_xq=False):
            b, qi = divmod(q, t_tiles // QUAD)
            if not skip_xq:
                xq = xq_pool.tile([P, QUAD, E], BF16, tag="xq")
                nc.sync.dma_start(xq[:], x16[b, :, QUAD * qi : QUAD * (qi + 1), :])
                xq_tiles[q] = xq
            xtq = xtq_pool.tile([P, QUAD, ET, P], FP8, tag="xtq")
            nc.sync.dma_start(xtq[:], xt8[b, :, QUAD * qi : QUAD * (qi + 1), :, :])
            xtq_tiles[q] = xtq

        fetch_quad(0, skip_xq=True)
        wet_t = []
        for pr in range(ET // 2):
            wt = const.tile([P, 2, D], FP8, name=f"wet{pr}")
            nc.sync.dma_start(wt[:], wet8[:, 2 * pr : 2 * pr + 2, :])
            wet_t.append(wt)
        z_t = [const.tile([P, D], BF16, name=f"z{b}") for b in range(bl)]
        nc.sync.dma_start(z_t[0][:], zrep16[:, 0])
        v_sb = const.tile([P, D], BF16)
        nc.sync.dma_start(v_sb[:], v16[:])
        xq0 = xq_pool.tile([P, QUAD, E], BF16, tag="xq")
        nc.sync.dma_start(xq0[:], x16[0, :, 0:QUAD, :])
        xq_tiles[0] = xq0
        onesc_sb = const.tile([P, 1], BF16)
        nc.sync.dma_start(onesc_sb[:], onesc[:])

        # ---- per-batch state ------------------------------------------------
        total = bl * t_tiles
        state = {}

        def get_state(b):
            if b not in state:
                state[b] = dict(
                    s_all=small.tile([P, t_tiles], F32, tag="s", name=f"s_all_{b}"),
                    s1_all=small.tile([P, t_tiles], F32, tag="s1", name=f"s1_all_{b}"),
                    p_all=small.tile([P, t_tiles], BF16, tag="p", name=f"p_all_{b}"),
                    ctx0=ps_ctx.tile([1, 512], F32, tag="ps_ctx", name=f"ctx0_{b}"),
                    ctx1=ps_ctx.tile([1, 512], F32, tag="ps_ctx", name=f"ctx1_{b}"),
                    proj_ps=[None] * t_tiles,
                    e_sbs=[None] * t_tiles,
                )
            return state[b]

        def emit_proj(b, i):
            # proj[t, d] = sum_e x[t, e] * 32*WeT[e, d], fp8 DoubleRow
            st = get_state(b)
            k = b * t_tiles + i
            q, qi = divmod(k, QUAD)
            xtq = xtq_tiles[q]
            pj = [
                ps_proj.tile([P, 512], F32, tag="ps_proj", name=f"pj{h}_{b}_{i}")
                for h in range(2)
            ]
            st["proj_ps"][i] = pj
            for pr in range(ET // 2):
                lhs = xtq[:, qi, 2 * pr : 2 * pr + 2, :]
                for h in range(2):
                    sl = slice(h * 512, (h + 1) * 512)
                    nc.tensor.matmul(
                        pj[h][:], lhs, wet_t[pr][:, :, sl],
                        start=(pr == 0), stop=(pr == ET // 2 - 1), perf_mode=DR,
                    )


        H = 384  # DVE v-dot on [0:H); Pool mult + ACT reduce on [H:D)

        def emit_epilogue(b, i):
            # energy = tanh((proj + 32z)/32); s = sum_d energy*v, split per
            # PSUM half so each stage starts as soon as its half is ready
            st = get_state(b)
            pj = st["proj_ps"][i]
            st["proj_ps"][i] = None
            e_sb = epool.tile([P, D], BF16, tag="e")
            st["e_sbs"][i] = e_sb
            for h in range(2):
                sl = slice(h * 512, (h + 1) * 512)
                nc.vector.tensor_add(e_sb[:, sl], pj[h][:], z_t[b][:, sl])
            nc.scalar.activation(
                e_sb[:], e_sb[:], mybir.ActivationFunctionType.Tanh,
                scale=1.0 / WSCALE,
            )
            nc.vector.scalar_tensor_tensor(
                out=e_sb[:, 0:H],
                in0=e_sb[:, 0:H],
                scalar=1.0,
                in1=v_sb[:, 0:H],
                op0=mybir.AluOpType.mult,
                op1=mybir.AluOpType.mult,
                accum_out=st["s_all"][:, i : i + 1],
            )
            nc.gpsimd.tensor_tensor(
                e_sb[:, H:D], e_sb[:, H:D], v_sb[:, H:D], mybir.AluOpType.mult
            )

        def emit_reduce(b, i):
            # s1 = sum of the Pool-multiplied half; deferred one step so the
            # ACT queue never waits on the Pool round-trip
            st = get_state(b)
            e_sb = st["e_sbs"][i]
            st["e_sbs"][i] = None
            nc.scalar.activation(
                e_sb[:, H:D], e_sb[:, H:D], mybir.ActivationFunctionType.Copy,
                accum_out=st["s1_all"][:, i : i + 1],
            )

        def emit_exp(b, i):
            # p = exp(s0 + s1); deferred so this ACT instr never blocks a tanh
            st = get_state(b)
            nc.scalar.activation(
                st["p_all"][:, i : i + 1],
                st["s_all"][:, i : i + 1],
                mybir.ActivationFunctionType.Exp,
                bias=st["s1_all"][:, i : i + 1],
            )

        def emit_ctx_half(b, i, h):
            # ctx_unnorm += p^T @ X  (contraction over the 128 t-rows), bf16
            st = get_state(b)
            k = b * t_tiles + i
            q, qi = divmod(k, QUAD)
            xq = xq_tiles[q]
            p_col = st["p_all"][:, i : i + 1]
            nc.tensor.matmul(
                st["ctx0" if h == 0 else "ctx1"][:], p_col,
                xq[:, qi, h * 512 : (h + 1) * 512],
                start=(i == 0), stop=(i == t_tiles - 1),
            )

        def emit_ctx(b, i):
            emit_ctx_half(b, i, 0)
            emit_ctx_half(b, i, 1)

        def emit_batch_end(b):
            # l = sum_t exp(s_t); ctx = ctx_unnorm / l.  The partition sum is
            # done via DMA-transpose + DVE reduce to keep it off the PE/PSUM.
            st = state.pop(b)
            l_part = small.tile([P, 1], BF16, tag="lp")
            with nc.allow_low_precision(reason="l partials; err ~0.2%/sqrt(128)"):
                nc.vector.tensor_reduce(
                    l_part[:], st["p_all"][:],
                    axis=mybir.AxisListType.X, op=mybir.AluOpType.add,
                )
            l_ps = ps_misc.tile([1, 1], F32, tag="ps_misc")
            nc.tensor.matmul(l_ps[:], l_part[:], onesc_sb[:])
            linv = small.tile([1, 1], F32, tag="linv")
            nc.vector.reciprocal(linv[:], l_ps[:])
            ctx_row = small.tile([1, E], F32, tag="ctxrow")
            nc.scalar.activation(
                ctx_row[:, 0:512], st["ctx0"][:],
                mybir.ActivationFunctionType.Copy, scale=linv[:],
            )
            nc.scalar.activation(
                ctx_row[:, 512:E], st["ctx1"][:],
                mybir.ActivationFunctionType.Copy, scale=linv[:],
            )
            nc.sync.dma_start(out[b : b + 1, :], ctx_row[:])

        # ---- main software pipeline over all (batch, t-tile) ----------------
        for k in range(total + CTX_LAG + END_LAG):
            if k < total:
                emit_proj(*divmod(k, t_tiles))
            if k == 0:
                for q in range(1, PREFETCH_QUADS):
                    fetch_quad(q)
            if k % QUAD == 0:
                qf = k // QUAD + PREFETCH_QUADS
                if qf < n_quads_total:
                    fetch_quad(qf)
            if k % t_tiles == 8 and k // t_tiles + 1 < bl:
                b_next = k // t_tiles + 1
                nc.sync.dma_start(z_t[b_next][:], zrep16[:, b_next])
            if 0 <= k - CTX_LAG < total:
                emit_ctx(*divmod(k - CTX_LAG, t_tiles))
            if 0 <= k - 1 < total:
                emit_epilogue(*divmod(k - 1, t_tiles))
            if 0 <= k - 2 < total:
                emit_reduce(*divmod(k - 2, t_tiles))
            if 0 <= k - EXP_LAG < total:
                emit_exp(*divmod(k - EXP_LAG, t_tiles))
            kb = k - CTX_LAG - END_LAG
            if 0 <= kb < total and kb % t_tiles == t_tiles - 1:
                emit_batch_end(kb // t_tiles)

    nc.compile()
    return nc


def _prep_inputs(enc_out, dec_state, W_weight, W_bias, v_weight, bl=BL):
    """Host-side layout/dtype prep + per-core slicing."""
    enc_out = np.ascontiguousarray(enc_out, dtype=np.float32)
    dec_state = np.ascontiguousarray(dec_state, dtype=np.float32)
    W = np.asarray(W_weight, dtype=np.float32)

    # x16: [B, tl, i, e] bf16
    x16_h = np.ascontiguousarray(
        enc_out.reshape(B, TT, P, E).transpose(0, 2, 1, 3).astype(NP_BF16)
    )
    # xt8: [B, el, i, j, tl] fp8
    enc8 = enc_out.astype(NP_FP8)
    xt8_h = np.ascontiguousarray(
        enc8.reshape(B, TT, P, ET, P).transpose(0, 4, 1, 3, 2)
    )
    # wet8: [el, j, d], scaled by WSCALE to avoid e4m3 subnormals
    wet8_h = np.ascontiguousarray(
        (WSCALE * W[:, :E].T).astype(NP_FP8).reshape(ET, P, D).transpose(1, 0, 2)
    )
    # z = Wd @ dec + bias, scaled by WSCALE, bf16, replicated over k
    z_all = dec_state @ W[:, E:].T + np.asarray(W_bias, dtype=np.float32)  # [B, D]
    z16 = (WSCALE * z_all).astype(NP_BF16)
    v16_h = np.ascontiguousarray(
        np.broadcast_to(np.asarray(v_weight).astype(NP_BF16).reshape(1, D), (P, D))
    )
    onesc_h = np.ones((P, 1), dtype=NP_BF16)

    in_maps = []
    for c in range(CORES):
        sl = slice(c * bl, (c + 1) * bl)
        zrep_h = np.ascontiguousarray(np.broadcast_to(z16[None, sl], (P, bl, D)))
        in_maps.append(
            {
                "x16": x16_h[sl],
                "xt8": xt8_h[sl],
                "wet8": wet8_h,
                "zrep16": zrep_h,
                "v16": v16_h,
                "onesc": onesc_h,
            }
        )
    return in_maps


_NC_CACHE = {}


def _get_nc():
    if "nc" not in _NC_CACHE:
        _NC_CACHE["nc"] = _build_kernel()
    return _NC_CACHE["nc"]


def _run(inputs, trace=False, tmpdir=None):
    nc = _get_nc()
    in_maps = _prep_inputs(
        inputs["enc_out"],
        inputs["dec_state"],
        inputs["W_weight"],
        inputs["W_bias"],
        inputs["v_weight"],
    )
    res = run_bass_kernel_spmd(
        nc, in_maps, list(range(CORES)), trace=trace, tmpdir=tmpdir
    )
    out = np.concatenate(
        [np.asarray(res.results[c]["ctx_out"]) for c in range(CORES)], axis=0
    )
    return out.astype(np.float32, copy=False), res


def kernel(**inputs):
    out, _ = _run(inputs, trace=False)
    return out


if __name__ == "__main__":
    pass



# revision 13
# speedup vs baseline: 1.0797x; 1.0797x over previous
"""Trainium2 Bass kernel for additive (Bahdanau) attention.

  context[b] = sum_t softmax_t( v . tanh(We @ enc[b,t] + Wd @ dec[b] + bias) ) * enc[b,t]

Shapes (hardcoded): enc_out [64, 2048, 1024] f32, dec_state [64, 1024] f32,
W_weight [1024, 2048], W_bias [1024], v_weight [1, 1024].  Output [64, 1024].

Sharding: data-parallel over batch across 8 NeuronCores (8 batches/core).

Design v3 (measured-rate rebalance of every engine; v1 in kernel_v1_baseline):
- Host prep: enc cast twice -- bf16 [b, tl, i, e] (ctx moving stream) and
  fp8-e4m3 [b, el, i, j, tl] (proj stationary).  We^T scaled x32 in fp8
  [el, j, d] pairs (dodges e4m3 subnormals; tanh's scale=1/32 undoes it).
  z = 32*(Wd @ dec + bias) bf16 replicated to 128 partitions.
- proj: 8 fp8 DoubleRow matmuls per tile into ONE [128,1024] f32 PSUM tile
  spanning 2 adjacent banks (ring of 3).  The z-add is then a single DVE
  tensor_add over all 1024 cols (measured 1.47us vs 1.63us for two).
- tanh runs on ACT over FOUR tiles at once ([128, 4096] in-place on a quad
  e-buffer): 0.98us/tile vs 1.36us single -- the per-instr overhead (~500ns)
  amortizes 4x.
- v-dot split by measured rates: DVE scalar_tensor_tensor w/accum fuses
  mult+reduce on cols [0:480] (1.27ns/col); Pool tensor_mul takes
  [480:1024] (2.0ns/col, idle engine) with ACT Copy+accum reducing that
  half (0.83ns/col + 0.5us fixed).  s = s0 + s1 via a tiny DVE add batched
  8 tiles wide, then one ACT exp per 8 tiles ([P,8]) -- exp cost drops
  from 336ns/tile to ~45ns/tile.
- ctx += p^T @ X: the two N=512 bf16 matmuls of a tile write partitions 0
  and 32 of the SAME PSUM bank = different PE column groups, so the PE
  overlaps them (tile_position auto-derives from the output base
  partition).  The batch's l-sum matmul rides partition 64.  ctx emits in
  2-tile groups to amortize fp8<->bf16 mode transitions.
- Softmax needs no max-subtraction (|scores| <= sum|v| <= 32).
"""
import os
import sys

sys.path.insert(0, "/opt/trn_rl_repo")

K_PJ2 = os.environ.get("K_PJ", "2") == "2"        # [128,1024] 2-bank pj tile
K_CTXCG = os.environ.get("K_CTX", "cg") == "cg"   # ctx col-group shared bank
K_TANHQ = os.environ.get("K_TANH", "q") == "q"    # quad tanh

from contextlib import ExitStack

import ml_dtypes
import numpy as np

import concourse.tile as tile
from concourse import bacc, mybir
from concourse.bass_utils import run_bass_kernel_spmd

F32 = mybir.dt.float32
BF16 = mybir.dt.bfloat16
FP8 = mybir.dt.float8e4
NP_FP8 = ml_dtypes.float8_e4m3
NP_BF16 = ml_dtypes.bfloat16
DR = mybir.MatmulPerfMode.DoubleRow

B, T, E, D = 64, 2048, 1024, 1024
CORES = 8
BL = B // CORES           # batches per core (8)
P = 128                   # partitions
TT = T // P               # t-tiles per batch (16)
ET = E // P               # e-blocks per row-tile (8)
QUAD = 4                  # t-tiles fetched per DMA instr / tanh'd per ACT instr
PREFETCH_QUADS = 4
WSCALE = 32.0             # fp8 subnormal-avoidance scale on We^T and z
ASPLIT = 480              # v-dot cols fused on DVE; rest Pool-mult + ACT-reduce

# pipeline lags (in t-tiles) behind emit_proj(k)
L_ZADD = 1                # DVE z-add of tile k-1 (quad-slice write)
# quad-tanh of tiles [k-4..k-1] fires when (k-1) completes a quad
L_VDOT = 5                # per-tile STT / Pool mult of tile k-5
L_RED = 6                 # ACT copy+accum of tile k-6
L_EXP = 7                 # s-add + exp of 4-group ending at k-7 (after its reduce)
L_CTX = 12                # ctx pair (k-12, k-11) on even kc
L_END = 14


def _build_kernel(bl=BL, t_tiles=TT):
    nc = bacc.Bacc(
        "TRN2",
        target_bir_lowering=False,
        debug=False,
        num_devices=CORES,
    )

    # [b, tl, i, e]: x16[b, tl, i, :] = enc[b, i*128+tl, :] in bf16
    x16 = nc.declare_dram_parameter("x16", [bl, P, t_tiles, E], BF16, isOutput=False)
    # [b, el, i, j, tl]: xt8[b, el, i, j, tl] = enc[b, i*128+tl, j*128+el] in fp8
    xt8 = nc.declare_dram_parameter("xt8", [bl, P, t_tiles, ET, P], FP8, isOutput=False)
    # [el, j, d]: wet8[el, j, d] = 32 * We[d, j*128+el] in fp8
    wet8 = nc.declare_dram_parameter("wet8", [P, ET, D], FP8, isOutput=False)
    # [k, b, d]: 32 * z[b, d] in bf16, replicated over k partitions
    zrep16 = nc.declare_dram_parameter("zrep16", [P, bl, D], BF16, isOutput=False)
    v16 = nc.declare_dram_parameter("v16", [P, D], BF16, isOutput=False)
    onesc = nc.declare_dram_parameter("onesc", [P, 1], BF16, isOutput=False)
    out = nc.declare_dram_parameter("ctx_out", [bl, E], F32, isOutput=True)

    n_quads_total = bl * t_tiles // QUAD

    with tile.TileContext(nc) as tc, ExitStack() as ctx:
        const = ctx.enter_context(tc.tile_pool(name="const", bufs=1))
        xq_pool = ctx.enter_context(tc.tile_pool(name="xq", bufs=6))
        xtq_pool = ctx.enter_context(tc.tile_pool(name="xtq", bufs=6))
        epool = ctx.enter_context(tc.tile_pool(name="e", bufs=3))
        small = ctx.enter_context(tc.tile_pool(name="small", bufs=2))

        n_proj_bufs = 3 if K_PJ2 else 5
        ps_proj = ctx.enter_context(
            tc.tile_pool(name="ps_proj", bufs=n_proj_bufs, space="PSUM"))
        ps_ctx = ctx.enter_context(tc.tile_pool(name="ps_ctx", bufs=2, space="PSUM"))
        if not (K_PJ2 and K_CTXCG):
            ps_misc = ctx.enter_context(
                tc.tile_pool(name="ps_misc", bufs=1, space="PSUM"))

        # ---- resident constants.  Ordered so proj(0)'s inputs land first.
        xq_tiles = {}
        xtq_tiles = {}

        def fetch_quad(q, skip_xq=False):
            b, qi = divmod(q, t_tiles // QUAD)
            if not skip_xq:
                xq = xq_pool.tile([P, QUAD, E], BF16, tag="xq")
                nc.sync.dma_start(xq[:], x16[b, :, QUAD * qi : QUAD * (qi + 1), :])
                xq_tiles[q] = xq
            xtq = xtq_pool.tile([P, QUAD, ET, P], FP8, tag="xtq")
            nc.sync.dma_start(xtq[:], xt8[b, :, QUAD * qi : QUAD * (qi + 1), :, :])
            xtq_tiles[q] = xtq

        fetch_quad(0, skip_xq=True)
        wet_t = []
        for pr in range(ET // 2):
            wt = const.tile([P, 2, D], FP8, name=f"wet{pr}")
            nc.sync.dma_start(wt[:], wet8[:, 2 * pr : 2 * pr + 2, :])
            wet_t.append(wt)
        z_t = [const.tile([P, D], BF16, name=f"z{b}") for b in range(bl)]
        nc.sync.dma_start(z_t[0][:], zrep16[:, 0])
        v_sb = const.tile([P, D], BF16)
        nc.sync.dma_start(v_sb[:], v16[:])
        xq0 = xq_pool.tile([P, QUAD, E], BF16, tag="xq")
        nc.sync.dma_start(xq0[:], x16[0, :, 0:QUAD, :])
        xq_tiles[0] = xq0
        onesc_sb = const.tile([P, 1], BF16)
        nc.sync.dma_start(onesc_sb[:], onesc[:])
        # scratch sinks (contents never read)
        junk = const.tile([P, ASPLIT], BF16, name="junk")
        prod = const.tile([P, 2, D - ASPLIT], BF16, name="prod")

        # ---- per-batch state ------------------------------------------------
        total = bl * t_tiles
        state = {}
        equads = {}   # quad index -> [128, QUAD, D] e-buffer

        def get_state(b):
            if b not in state:
                state[b] = dict(
                    s0=small.tile([P, t_tiles], F32, tag="s0", name=f"s0_{b}"),
                    s1=small.tile([P, t_tiles], F32, tag="s1", name=f"s1_{b}"),
                    s=small.tile([P, t_tiles], F32, tag="s", name=f"s_{b}"),
                    p_all=small.tile([P, t_tiles], BF16, tag="p", name=f"p_all_{b}"),
                    # one PSUM bank: ctx halves at partitions 0 / 32, l at 64
                    ctxb=(ps_ctx.tile([P, 512], F32, tag="ps_ctx", name=f"ctxb_{b}")
                          if K_CTXCG else None),
                    ctx0=(None if K_CTXCG else
                          ps_ctx.tile([1, 512], F32, tag="ps_ctx", name=f"ctx0_{b}")),
                    ctx1=(None if K_CTXCG else
                          ps_ctx.tile([1, 512], F32, tag="ps_ctx", name=f"ctx1_{b}")),
                    proj_ps=[None] * t_tiles,
                )
            return state[b]

        def emit_proj(b, i):
            # proj[t, d] = sum_e x[t, e] * 32*WeT[e, d], fp8 DoubleRow,
            # both 512-halves into one [128,1024] 2-bank PSUM tile
            st = get_state(b)
            k = b * t_tiles + i
            q, qi = divmod(k, QUAD)
            xtq = xtq_tiles[q]
            if K_PJ2:
                pj = ps_proj.tile([P, D], F32, tag="ps_proj", name=f"pj_{b}_{i}")
                pjs = [pj[:, 0:512], pj[:, 512:1024]]
                st["proj_ps"][i] = [pj]
            else:
                pj0 = ps_proj.tile([P, 512], F32, tag="ps_proj", name=f"pj0_{b}_{i}")
                pj1 = ps_proj.tile([P, 512], F32, tag="ps_proj", name=f"pj1_{b}_{i}")
                pjs = [pj0[:], pj1[:]]
                st["proj_ps"][i] = [pj0, pj1]
            for pr in range(ET // 2):
                lhs = xtq[:, qi, 2 * pr : 2 * pr + 2, :]
                for h in range(2):
                    nc.tensor.matmul(
                        pjs[h], lhs, wet_t[pr][:, :, 512 * h : 512 * (h + 1)],
                        start=(pr == 0), stop=(pr == ET // 2 - 1), perf_mode=DR,
                    )

        def emit_zadd(k):
            # e = (proj + 32z) bf16 into this quad's slice; one [128,1024]
            # DVE op spanning both PSUM banks
            b, i = divmod(k, t_tiles)
            st = get_state(b)
            q, qi = divmod(k, QUAD)
            if q not in equads:
                equads[q] = epool.tile([P, QUAD, D], BF16, tag="e", name=f"equad_{q}")
            pjl = st["proj_ps"][i]
            st["proj_ps"][i] = None
            if K_PJ2:
                nc.vector.tensor_add(equads[q][:, qi, :], pjl[0][:], z_t[b][:])
            else:
                for h in range(2):
                    sl = slice(512 * h, 512 * (h + 1))
                    nc.vector.tensor_add(
                        equads[q][:, qi, sl], pjl[h][:], z_t[b][:, sl]
                    )

        def emit_tanh_quad(q):
            # tanh over 4 tiles at once; ~0.98us/tile vs 1.36 single
            if K_TANHQ:
                nc.scalar.activation(
                    equads[q][:], equads[q][:], mybir.ActivationFunctionType.Tanh,
                    scale=1.0 / WSCALE,
                )
            else:
                for qi in range(QUAD):
                    nc.scalar.activation(
                        equads[q][:, qi, :], equads[q][:, qi, :],
                        mybir.ActivationFunctionType.Tanh, scale=1.0 / WSCALE,
                    )

        def emit_vdot(k):
            # cols [0:ASPLIT): DVE STT fused mult+accum -> s0
            # cols [ASPLIT:D): Pool mult -> prod (ACT reduces next step)
            b, i = divmod(k, t_tiles)
            st = get_state(b)
            q, qi = divmod(k, QUAD)
            e_sb = equads[q]
            nc.vector.scalar_tensor_tensor(
                out=junk[:],
                in0=e_sb[:, qi, 0:ASPLIT],
                scalar=1.0,
                in1=v_sb[:, 0:ASPLIT],
                op0=mybir.AluOpType.mult,
                op1=mybir.AluOpType.mult,
                accum_out=st["s0"][:, i : i + 1],
            )
            nc.gpsimd.tensor_mul(
                prod[:, k % 2, :], e_sb[:, qi, ASPLIT:D], v_sb[:, ASPLIT:D]
            )

        def emit_reduce(k):
            # ACT Copy+accum over Pool's product half -> s1
            b, i = divmod(k, t_tiles)
            st = get_state(b)
            nc.scalar.activation(
                prod[:, k % 2, :],
                prod[:, k % 2, :],
                mybir.ActivationFunctionType.Copy,
                accum_out=st["s1"][:, i : i + 1],
            )

        def emit_exp4(b, i0):
            # s = s0+s1 (DVE, [P,4]) then p = exp(s) (one ACT instr)
            st = get_state(b)
            sl = slice(i0, i0 + 4)
            nc.vector.tensor_add(st["s"][:, sl], st["s0"][:, sl], st["s1"][:, sl])
            nc.scalar.activation(
                st["p_all"][:, sl], st["s"][:, sl],
                mybir.ActivationFunctionType.Exp,
            )

        def emit_ctx(b, i):
            # ctx_unnorm += p^T @ X; halves go to partitions 0 / 32 of one
            # PSUM bank = different PE column groups -> concurrent
            st = get_state(b)
            k = b * t_tiles + i
            q, qi = divmod(k, QUAD)
            xq = xq_tiles[q]
            p_col = st["p_all"][:, i : i + 1]
            for h in range(2):
                if K_CTXCG:
                    dst = st["ctxb"][32 * h : 32 * h + 1, :]
                else:
                    dst = st["ctx0" if h == 0 else "ctx1"][:]
                nc.tensor.matmul(
                    dst, p_col,
                    xq[:, qi, h * 512 : (h + 1) * 512],
                    start=(i == 0), stop=(i == t_tiles - 1),
                )

        def emit_batch_end(b):
            # l = sum_t exp(s_t); ctx = ctx_unnorm / l.  Partition sum via
            # DVE reduce + 1-col matmul into partition 64 of the ctx bank.
            st = state.pop(b)
            l_part = small.tile([P, 1], BF16, tag="lp")
            with nc.allow_low_precision(reason="l partials; err ~0.2%/sqrt(128)"):
                nc.vector.tensor_reduce(
                    l_part[:], st["p_all"][:],
                    axis=mybir.AxisListType.X, op=mybir.AluOpType.add,
                )
            if K_CTXCG:
                l_ps = st["ctxb"][64:65, 0:1]
                c0, c1 = st["ctxb"][0:1, :], st["ctxb"][32:33, :]
            else:
                lt = ps_misc.tile([1, 1], F32, tag="ps_misc", name=f"l_{b}")
                l_ps = lt[:]
                c0, c1 = st["ctx0"][:], st["ctx1"][:]
            nc.tensor.matmul(l_ps, l_part[:], onesc_sb[:])
            linv = small.tile([1, 1], F32, tag="linv")
            nc.vector.reciprocal(linv[:], l_ps)
            ctx_row = small.tile([1, E], F32, tag="ctxrow")
            nc.scalar.activation(
                ctx_row[:, 0:512], c0,
                mybir.ActivationFunctionType.Copy, scale=linv[:],
            )
            nc.scalar.activation(
                ctx_row[:, 512:E], c1,
                mybir.ActivationFunctionType.Copy, scale=linv[:],
            )
            nc.sync.dma_start(out[b : b + 1, :], ctx_row[:])

        # ---- main software pipeline over all (batch, t-tile) ----------------
        for k in range(total + L_END + 2):
            if k < total:
                emit_proj(*divmod(k, t_tiles))
            if k == 0:
                for q in range(1, PREFETCH_QUADS):
                    fetch_quad(q)
            if k % QUAD == 0:
                qf = k // QUAD + PREFETCH_QUADS
                if qf < n_quads_total:
                    fetch_quad(qf)
            if k % t_tiles == 8 and k // t_tiles + 1 < bl:
                b_next = k // t_tiles + 1
                nc.sync.dma_start(z_t[b_next][:], zrep16[:, b_next])
            kz = k - L_ZADD
            if 0 <= kz < total:
                emit_zadd(kz)
                if kz % QUAD == QUAD - 1:
                    emit_tanh_quad(kz // QUAD)
            kv = k - L_VDOT
            if 0 <= kv < total:
                emit_vdot(kv)
            kr = k - L_RED
            if 0 <= kr < total:
                emit_reduce(kr)
            ke = k - L_EXP
            if 0 <= ke < total and ke % 4 == 3:
                b_e, i_e = divmod(ke, t_tiles)
                emit_exp4(b_e, (i_e // 4) * 4)
            kc = k - L_CTX
            if kc >= 0 and kc % 2 == 0:
                for kk in (kc, kc + 1):
                    if 0 <= kk < total:
                        emit_ctx(*divmod(kk, t_tiles))
            kb = k - L_END
            if 0 <= kb < total and kb % t_tiles == t_tiles - 1:
                emit_batch_end(kb // t_tiles)
            # free the consumed e-quad once its last tile's reduce is done
            kq = k - L_RED - 1
            if kq >= 0 and kq % QUAD == QUAD - 1 and (kq // QUAD) in equads:
                equads.pop(kq // QUAD)

    nc.compile()
    return nc


def _prep_inputs(enc_out, dec_state, W_weight, W_bias, v_weight, bl=BL):
    """Host-side layout/dtype prep + per-core slicing."""
    enc_out = np.ascontiguousarray(enc_out, dtype=np.float32)
    dec_state = np.ascontiguousarray(dec_state, dtype=np.float32)
    W = np.asarray(W_weight, dtype=np.float32)

    # x16: [B, tl, i, e] bf16
    x16_h = np.ascontiguousarray(
        enc_out.reshape(B, TT, P, E).transpose(0, 2, 1, 3).astype(NP_BF16)
    )
    # xt8: [B, el, i, j, tl] fp8
    enc8 = enc_out.astype(NP_FP8)
    xt8_h = np.ascontiguousarray(
        enc8.reshape(B, TT, P, ET, P).transpose(0, 4, 1, 3, 2)
    )
    # wet8: [el, j, d], scaled by WSCALE to avoid e4m3 subnormals
    wet8_h = np.ascontiguousarray(
        (WSCALE * W[:, :E].T).astype(NP_FP8).reshape(ET, P, D).transpose(1, 0, 2)
    )
    # z = Wd @ dec + bias, scaled by WSCALE, bf16, replicated over k
    z_all = dec_state @ W[:, E:].T + np.asarray(W_bias, dtype=np.float32)  # [B, D]
    z16 = (WSCALE * z_all).astype(NP_BF16)
    v16_h = np.ascontiguousarray(
        np.broadcast_to(np.asarray(v_weight).astype(NP_BF16).reshape(1, D), (P, D))
    )
    onesc_h = np.ones((P, 1), dtype=NP_BF16)

    in_maps = []
    for c in range(CORES):
        sl = slice(c * bl, (c + 1) * bl)
        zrep_h = np.ascontiguousarray(np.broadcast_to(z16[None, sl], (P, bl, D)))
        in_maps.append(
            {
                "x16": x16_h[sl],
                "xt8": xt8_h[sl],
                "wet8": wet8_h,
                "zrep16": zrep_h,
                "v16": v16_h,
                "onesc": onesc_h,
            }
        )
    return in_maps


_NC_CACHE = {}


def _get_nc():
    if "nc" not in _NC_CACHE:
        _NC_CACHE["nc"] = _build_kernel()
    return _NC_CACHE["nc"]


def _run(inputs, trace=False, tmpdir=None):
    nc = _get_nc()
    in_maps = _prep_inputs(
        inputs["enc_out"],
        inputs["dec_state"],
        inputs["W_weight"],
        inputs["W_bias"],
        inputs["v_weight"],
    )
    res = run_bass_kernel_spmd(
        nc, in_maps, list(range(CORES)), trace=trace, tmpdir=tmpdir
    )
    out = np.concatenate(
        [np.asarray(res.results[c]["ctx_out"]) for c in range(CORES)], axis=0
    )
    return out.astype(np.float32, copy=False), res


def kernel(**inputs):
    out, _ = _run(inputs, trace=False)
    return out


if __name__ == "__main__":
    pass


# revision 14
# speedup vs baseline: 1.1469x; 1.0622x over previous
"""Trainium2 Bass kernel for additive (Bahdanau) attention.

  context[b] = sum_t softmax_t( v . tanh(We @ enc[b,t] + Wd @ dec[b] + bias) ) * enc[b,t]

Shapes (hardcoded): enc_out [64, 2048, 1024] f32, dec_state [64, 1024] f32,
W_weight [1024, 2048], W_bias [1024], v_weight [1, 1024].  Output [64, 1024].

Sharding: data-parallel over batch across 8 NeuronCores (8 batches/core).

Design v3 (measured-rate rebalance of every engine; v1 in kernel_v1_baseline):
- Host prep: enc cast twice -- bf16 [b, tl, i, e] (ctx moving stream) and
  fp8-e4m3 [b, el, i, j, tl] (proj stationary).  We^T scaled x32 in fp8
  [el, j, d] pairs (dodges e4m3 subnormals; tanh's scale=1/32 undoes it).
  z = 32*(Wd @ dec + bias) bf16 replicated to 128 partitions.
- proj: 8 fp8 DoubleRow matmuls per tile into ONE [128,1024] f32 PSUM tile
  spanning 2 adjacent banks (ring of 3).  The z-add is then a single DVE
  tensor_add over all 1024 cols (measured 1.47us vs 1.63us for two).
- tanh runs on ACT over FOUR tiles at once ([128, 4096] in-place on a quad
  e-buffer): 0.98us/tile vs 1.36us single -- the per-instr overhead (~500ns)
  amortizes 4x.
- v-dot split by measured rates: DVE scalar_tensor_tensor w/accum fuses
  mult+reduce on cols [0:480] (1.27ns/col); Pool tensor_mul takes
  [480:1024] (2.0ns/col, idle engine) with ACT Copy+accum reducing that
  half (0.83ns/col + 0.5us fixed).  s = s0 + s1 via a tiny DVE add batched
  8 tiles wide, then one ACT exp per 8 tiles ([P,8]) -- exp cost drops
  from 336ns/tile to ~45ns/tile.
- ctx += p^T @ X: the two N=512 bf16 matmuls of a tile write partitions 0
  and 32 of the SAME PSUM bank = different PE column groups, so the PE
  overlaps them (tile_position auto-derives from the output base
  partition).  The batch's l-sum matmul rides partition 64.  ctx emits in
  2-tile groups to amortize fp8<->bf16 mode transitions.
- Softmax needs no max-subtraction (|scores| <= sum|v| <= 32).
"""
import os
import sys

sys.path.insert(0, "/opt/trn_rl_repo")

K_PJ2 = os.environ.get("K_PJ", "2") == "2"        # [128,1024] 2-bank pj tile
K_CTXCG = os.environ.get("K_CTX", "cg") == "cg"   # ctx col-group shared bank
K_TANHQ = os.environ.get("K_TANH", "q") == "q"    # quad tanh

from contextlib import ExitStack

import ml_dtypes
import numpy as np

import concourse.tile as tile
from concourse import bacc, mybir
from concourse.bass_utils import run_bass_kernel_spmd

F32 = mybir.dt.float32
BF16 = mybir.dt.bfloat16
FP8 = mybir.dt.float8e4
NP_FP8 = ml_dtypes.float8_e4m3
NP_BF16 = ml_dtypes.bfloat16
DR = mybir.MatmulPerfMode.DoubleRow

B, T, E, D = 64, 2048, 1024, 1024
CORES = 8
BL = B // CORES           # batches per core (8)
P = 128                   # partitions
TT = T // P               # t-tiles per batch (16)
ET = E // P               # e-blocks per row-tile (8)
QUAD = 4                  # t-tiles fetched per DMA instr / tanh'd per ACT instr
PREFETCH_QUADS = 4
WSCALE = 32.0             # fp8 subnormal-avoidance scale on We^T and z
ASPLIT = 480              # v-dot cols fused on DVE; rest Pool-mult + ACT-reduce

# pipeline lags (in t-tiles) behind emit_proj(k)
L_ZADD = 1                # DVE z-add of tile k-1 (quad-slice write)
# quad-tanh of tiles [k-4..k-1] fires when (k-1) completes a quad
L_VDOT = 5                # per-tile STT / Pool mult of tile k-5
L_RED = 6                 # ACT copy+accum of tile k-6
L_EXP = 7                 # s-add + exp of 4-group ending at k-7 (after its reduce)
L_CTX = 12                # ctx pair (k-12, k-11) on even kc
L_END = 14


def _build_kernel(bl=BL, t_tiles=TT):
    nc = bacc.Bacc(
        "TRN2",
        target_bir_lowering=False,
        debug=False,
        num_devices=CORES,
    )

    # [b, tl, i, e]: x16[b, tl, i, :] = enc[b, i*128+tl, :] in bf16
    x16 = nc.declare_dram_parameter("x16", [bl, P, t_tiles, E], BF16, isOutput=False)
    # [b, el, i, j, tl]: xt8[b, el, i, j, tl] = enc[b, i*128+tl, j*128+el] in fp8
    xt8 = nc.declare_dram_parameter("xt8", [bl, P, t_tiles, ET, P], FP8, isOutput=False)
    # [el, j, d]: wet8[el, j, d] = 32 * We[d, j*128+el] in fp8
    wet8 = nc.declare_dram_parameter("wet8", [P, ET, D], FP8, isOutput=False)
    # [k, b, d]: 32 * z[b, d] in bf16, replicated over k partitions
    zrep16 = nc.declare_dram_parameter("zrep16", [P, bl, D], BF16, isOutput=False)
    v16 = nc.declare_dram_parameter("v16", [P, D], BF16, isOutput=False)
    onesc = nc.declare_dram_parameter("onesc", [P, 1], BF16, isOutput=False)
    out = nc.declare_dram_parameter("ctx_out", [bl, E], F32, isOutput=True)

    n_quads_total = bl * t_tiles // QUAD

    with tile.TileContext(nc) as tc, ExitStack() as ctx:
        const = ctx.enter_context(tc.tile_pool(name="const", bufs=1))
        xq_pool = ctx.enter_context(tc.tile_pool(name="xq", bufs=6))
        xtq_pool = ctx.enter_context(tc.tile_pool(name="xtq", bufs=6))
        epool = ctx.enter_context(tc.tile_pool(name="e", bufs=5))
        small = ctx.enter_context(tc.tile_pool(name="small", bufs=2))

        n_proj_bufs = 3 if K_PJ2 else 5
        ps_proj = ctx.enter_context(
            tc.tile_pool(name="ps_proj", bufs=n_proj_bufs, space="PSUM"))
        ps_ctx = ctx.enter_context(tc.tile_pool(name="ps_ctx", bufs=2, space="PSUM"))
        if not (K_PJ2 and K_CTXCG):
            ps_misc = ctx.enter_context(
                tc.tile_pool(name="ps_misc", bufs=1, space="PSUM"))

        # ---- resident constants.  Ordered so proj(0)'s inputs land first.
        xq_tiles = {}
        xtq_tiles = {}

        def fetch_quad(q, skip_xq=False):
            b, qi = divmod(q, t_tiles // QUAD)
            if not skip_xq:
                xq = xq_pool.tile([P, QUAD, E], BF16, tag="xq")
                nc.sync.dma_start(xq[:], x16[b, :, QUAD * qi : QUAD * (qi + 1), :])
                xq_tiles[q] = xq
            xtq = xtq_pool.tile([P, QUAD, ET, P], FP8, tag="xtq")
            nc.sync.dma_start(xtq[:], xt8[b, :, QUAD * qi : QUAD * (qi + 1), :, :])
            xtq_tiles[q] = xtq

        fetch_quad(0, skip_xq=True)
        wet_t = []
        for pr in range(ET // 2):
            wt = const.tile([P, 2, D], FP8, name=f"wet{pr}")
            nc.sync.dma_start(wt[:], wet8[:, 2 * pr : 2 * pr + 2, :])
            wet_t.append(wt)
        z_t = [const.tile([P, D], BF16, name=f"z{b}") for b in range(bl)]
        nc.sync.dma_start(z_t[0][:], zrep16[:, 0])
        v_sb = const.tile([P, D], BF16)
        nc.sync.dma_start(v_sb[:], v16[:])
        xq0 = xq_pool.tile([P, QUAD, E], BF16, tag="xq")
        nc.sync.dma_start(xq0[:], x16[0, :, 0:QUAD, :])
        xq_tiles[0] = xq0
        onesc_sb = const.tile([P, 1], BF16)
        nc.sync.dma_start(onesc_sb[:], onesc[:])
        # scratch sinks (contents never read)
        junk = const.tile([P, ASPLIT], BF16, name="junk")
        prod = const.tile([P, 4, D - ASPLIT], BF16, name="prod")

        # ---- per-batch state ------------------------------------------------
        total = bl * t_tiles
        state = {}
        equads = {}   # quad index -> [128, QUAD, D] e-buffer

        def get_state(b):
            if b not in state:
                state[b] = dict(
                    s0=small.tile([P, t_tiles], F32, tag="s0", name=f"s0_{b}"),
                    s1=small.tile([P, t_tiles], F32, tag="s1", name=f"s1_{b}"),
                    s=small.tile([P, t_tiles], F32, tag="s", name=f"s_{b}"),
                    p_all=small.tile([P, t_tiles], BF16, tag="p", name=f"p_all_{b}"),
                    # one PSUM bank: ctx halves at partitions 0 / 32, l at 64
                    ctxb=(ps_ctx.tile([P, 512], F32, tag="ps_ctx", name=f"ctxb_{b}")
                          if K_CTXCG else None),
                    ctx0=(None if K_CTXCG else
                          ps_ctx.tile([1, 512], F32, tag="ps_ctx", name=f"ctx0_{b}")),
                    ctx1=(None if K_CTXCG else
                          ps_ctx.tile([1, 512], F32, tag="ps_ctx", name=f"ctx1_{b}")),
                    proj_ps=[None] * t_tiles,
                )
            return state[b]

        def emit_proj(b, i):
            # proj[t, d] = sum_e x[t, e] * 32*WeT[e, d], fp8 DoubleRow,
            # both 512-halves into one [128,1024] 2-bank PSUM tile
            st = get_state(b)
            k = b * t_tiles + i
            q, qi = divmod(k, QUAD)
            xtq = xtq_tiles[q]
            if K_PJ2:
                pj = ps_proj.tile([P, D], F32, tag="ps_proj", name=f"pj_{b}_{i}")
                pjs = [pj[:, 0:512], pj[:, 512:1024]]
                st["proj_ps"][i] = [pj]
            else:
                pj0 = ps_proj.tile([P, 512], F32, tag="ps_proj", name=f"pj0_{b}_{i}")
                pj1 = ps_proj.tile([P, 512], F32, tag="ps_proj", name=f"pj1_{b}_{i}")
                pjs = [pj0[:], pj1[:]]
                st["proj_ps"][i] = [pj0, pj1]
            for pr in range(ET // 2):
                lhs = xtq[:, qi, 2 * pr : 2 * pr + 2, :]
                for h in range(2):
                    nc.tensor.matmul(
                        pjs[h], lhs, wet_t[pr][:, :, 512 * h : 512 * (h + 1)],
                        start=(pr == 0), stop=(pr == ET // 2 - 1), perf_mode=DR,
                    )

        def emit_zadd(k):
            # e = (proj + 32z) bf16 into this quad's slice; one [128,1024]
            # DVE op spanning both PSUM banks
            b, i = divmod(k, t_tiles)
            st = get_state(b)
            q, qi = divmod(k, QUAD)
            if q not in equads:
                equads[q] = epool.tile([P, QUAD, D], BF16, tag="e", name=f"equad_{q}")
            pjl = st["proj_ps"][i]
            st["proj_ps"][i] = None
            if K_PJ2:
                nc.vector.tensor_add(equads[q][:, qi, :], pjl[0][:], z_t[b][:])
            else:
                for h in range(2):
                    sl = slice(512 * h, 512 * (h + 1))
                    nc.vector.tensor_add(
                        equads[q][:, qi, sl], pjl[h][:], z_t[b][:, sl]
                    )

        def emit_tanh_quad(q):
            # tanh over 4 tiles at once; ~0.98us/tile vs 1.36 single
            if K_TANHQ:
                nc.scalar.activation(
                    equads[q][:], equads[q][:], mybir.ActivationFunctionType.Tanh,
                    scale=1.0 / WSCALE,
                )
            else:
                for qi in range(QUAD):
                    nc.scalar.activation(
                        equads[q][:, qi, :], equads[q][:, qi, :],
                        mybir.ActivationFunctionType.Tanh, scale=1.0 / WSCALE,
                    )

        def emit_vdot(k):
            # cols [0:ASPLIT): DVE STT fused mult+accum -> s0
            # cols [ASPLIT:D): Pool mult -> prod (ACT reduces next step)
            b, i = divmod(k, t_tiles)
            st = get_state(b)
            q, qi = divmod(k, QUAD)
            e_sb = equads[q]
            nc.vector.scalar_tensor_tensor(
                out=junk[:],
                in0=e_sb[:, qi, 0:ASPLIT],
                scalar=1.0,
                in1=v_sb[:, 0:ASPLIT],
                op0=mybir.AluOpType.mult,
                op1=mybir.AluOpType.mult,
                accum_out=st["s0"][:, i : i + 1],
            )
            nc.gpsimd.tensor_mul(
                prod[:, k % 4, :], e_sb[:, qi, ASPLIT:D], v_sb[:, ASPLIT:D]
            )

        def emit_reduce(k):
            # ACT Copy+accum over Pool's product half -> s1
            b, i = divmod(k, t_tiles)
            st = get_state(b)
            nc.scalar.activation(
                prod[:, k % 4, :],
                prod[:, k % 4, :],
                mybir.ActivationFunctionType.Copy,
                accum_out=st["s1"][:, i : i + 1],
            )

        def emit_exp4(b, i0):
            # s = s0+s1 (DVE, [P,4]) then p = exp(s) (one ACT instr)
            st = get_state(b)
            sl = slice(i0, i0 + 4)
            nc.vector.tensor_add(st["s"][:, sl], st["s0"][:, sl], st["s1"][:, sl])
            nc.scalar.activation(
                st["p_all"][:, sl], st["s"][:, sl],
                mybir.ActivationFunctionType.Exp,
            )

        def emit_ctx(b, i):
            # ctx_unnorm += p^T @ X; halves go to partitions 0 / 32 of one
            # PSUM bank = different PE column groups -> concurrent
            st = get_state(b)
            k = b * t_tiles + i
            q, qi = divmod(k, QUAD)
            xq = xq_tiles[q]
            p_col = st["p_all"][:, i : i + 1]
            for h in range(2):
                if K_CTXCG:
                    dst = st["ctxb"][32 * h : 32 * h + 1, :]
                else:
                    dst = st["ctx0" if h == 0 else "ctx1"][:]
                nc.tensor.matmul(
                    dst, p_col,
                    xq[:, qi, h * 512 : (h + 1) * 512],
                    start=(i == 0), stop=(i == t_tiles - 1),
                )

        def emit_batch_end(b):
            # l = sum_t exp(s_t); ctx = ctx_unnorm / l.  Partition sum via
            # DVE reduce + 1-col matmul into partition 64 of the ctx bank.
            st = state.pop(b)
            l_part = small.tile([P, 1], BF16, tag="lp")
            with nc.allow_low_precision(reason="l partials; err ~0.2%/sqrt(128)"):
                nc.vector.tensor_reduce(
                    l_part[:], st["p_all"][:],
                    axis=mybir.AxisListType.X, op=mybir.AluOpType.add,
                )
            if K_CTXCG:
                l_ps = st["ctxb"][64:65, 0:1]
                c0, c1 = st["ctxb"][0:1, :], st["ctxb"][32:33, :]
            else:
                lt = ps_misc.tile([1, 1], F32, tag="ps_misc", name=f"l_{b}")
                l_ps = lt[:]
                c0, c1 = st["ctx0"][:], st["ctx1"][:]
            nc.tensor.matmul(l_ps, l_part[:], onesc_sb[:])
            linv = small.tile([1, 1], F32, tag="linv")
            nc.vector.reciprocal(linv[:], l_ps)
            ctx_row = small.tile([1, E], F32, tag="ctxrow")
            nc.scalar.activation(
                ctx_row[:, 0:512], c0,
                mybir.ActivationFunctionType.Copy, scale=linv[:],
            )
            nc.scalar.activation(
                ctx_row[:, 512:E], c1,
                mybir.ActivationFunctionType.Copy, scale=linv[:],
            )
            nc.sync.dma_start(out[b : b + 1, :], ctx_row[:])

        # ---- main software pipeline over all (batch, t-tile) ----------------
        for k in range(total + L_END + 2):
            if k < total:
                emit_proj(*divmod(k, t_tiles))
            if k == 0:
                for q in range(1, PREFETCH_QUADS):
                    fetch_quad(q)
            if k % QUAD == 0:
                qf = k // QUAD + PREFETCH_QUADS
                if qf < n_quads_total:
                    fetch_quad(qf)
            if k % t_tiles == 8 and k // t_tiles + 1 < bl:
                b_next = k // t_tiles + 1
                nc.sync.dma_start(z_t[b_next][:], zrep16[:, b_next])
            kz = k - L_ZADD
            if 0 <= kz < total:
                emit_zadd(kz)
                if kz % QUAD == QUAD - 1:
                    emit_tanh_quad(kz // QUAD)
            kv = k - L_VDOT
            if 0 <= kv < total:
                emit_vdot(kv)
            kr = k - L_RED
            if 0 <= kr < total:
                emit_reduce(kr)
            ke = k - L_EXP
            if 0 <= ke < total and ke % 4 == 3:
                b_e, i_e = divmod(ke, t_tiles)
                emit_exp4(b_e, (i_e // 4) * 4)
            kc = k - L_CTX
            if kc >= 0 and kc % 2 == 0:
                for kk in (kc, kc + 1):
                    if 0 <= kk < total:
                        emit_ctx(*divmod(kk, t_tiles))
            kb = k - L_END
            if 0 <= kb < total and kb % t_tiles == t_tiles - 1:
                emit_batch_end(kb // t_tiles)
            # free the consumed e-quad once its last tile's reduce is done
            kq = k - L_RED - 1
            if kq >= 0 and kq % QUAD == QUAD - 1 and (kq // QUAD) in equads:
                equads.pop(kq // QUAD)

    nc.compile()
    return nc


def _prep_inputs(enc_out, dec_state, W_weight, W_bias, v_weight, bl=BL):
    """Host-side layout/dtype prep + per-core slicing."""
    enc_out = np.ascontiguousarray(enc_out, dtype=np.float32)
    dec_state = np.ascontiguousarray(dec_state, dtype=np.float32)
    W = np.asarray(W_weight, dtype=np.float32)

    # x16: [B, tl, i, e] bf16
    x16_h = np.ascontiguousarray(
        enc_out.reshape(B, TT, P, E).transpose(0, 2, 1, 3).astype(NP_BF16)
    )
    # xt8: [B, el, i, j, tl] fp8
    enc8 = enc_out.astype(NP_FP8)
    xt8_h = np.ascontiguousarray(
        enc8.reshape(B, TT, P, ET, P).transpose(0, 4, 1, 3, 2)
    )
    # wet8: [el, j, d], scaled by WSCALE to avoid e4m3 subnormals
    wet8_h = np.ascontiguousarray(
        (WSCALE * W[:, :E].T).astype(NP_FP8).reshape(ET, P, D).transpose(1, 0, 2)
    )
    # z = Wd @ dec + bias, scaled by WSCALE, bf16, replicated over k
    z_all = dec_state @ W[:, E:].T + np.asarray(W_bias, dtype=np.float32)  # [B, D]
    z16 = (WSCALE * z_all).astype(NP_BF16)
    v16_h = np.ascontiguousarray(
        np.broadcast_to(np.asarray(v_weight).astype(NP_BF16).reshape(1, D), (P, D))
    )
    onesc_h = np.ones((P, 1), dtype=NP_BF16)

    in_maps = []
    for c in range(CORES):
        sl = slice(c * bl, (c + 1) * bl)
        zrep_h = np.ascontiguousarray(np.broadcast_to(z16[None, sl], (P, bl, D)))
        in_maps.append(
            {
                "x16": x16_h[sl],
                "xt8": xt8_h[sl],
                "wet8": wet8_h,
                "zrep16": zrep_h,
                "v16": v16_h,
                "onesc": onesc_h,
            }
        )
    return in_maps


_NC_CACHE = {}


def _get_nc():
    if "nc" not in _NC_CACHE:
        _NC_CACHE["nc"] = _build_kernel()
    return _NC_CACHE["nc"]


def _run(inputs, trace=False, tmpdir=None):
    nc = _get_nc()
    in_maps = _prep_inputs(
        inputs["enc_out"],
        inputs["dec_state"],
        inputs["W_weight"],
        inputs["W_bias"],
        inputs["v_weight"],
    )
    res = run_bass_kernel_spmd(
        nc, in_maps, list(range(CORES)), trace=trace, tmpdir=tmpdir
    )
    out = np.concatenate(
        [np.asarray(res.results[c]["ctx_out"]) for c in range(CORES)], axis=0
    )
    return out.astype(np.float32, copy=False), res


def kernel(**inputs):
    out, _ = _run(inputs, trace=False)
    return out


if __name__ == "__main__":
    pass


# revision 15
# speedup vs baseline: 1.1780x; 1.0272x over previous
"""Trainium2 Bass kernel for additive (Bahdanau) attention.

  context[b] = sum_t softmax_t( v . tanh(We @ enc[b,t] + Wd @ dec[b] + bias) ) * enc[b,t]

Shapes (hardcoded): enc_out [64, 2048, 1024] f32, dec_state [64, 1024] f32,
W_weight [1024, 2048], W_bias [1024], v_weight [1, 1024].  Output [64, 1024].

Sharding: data-parallel over batch across 8 NeuronCores (8 batches/core).

Design v3 (measured-rate rebalance of every engine; v1 in kernel_v1_baseline):
- Host prep: enc cast twice -- bf16 [b, tl, i, e] (ctx moving stream) and
  fp8-e4m3 [b, el, i, j, tl] (proj stationary).  We^T scaled x32 in fp8
  [el, j, d] pairs (dodges e4m3 subnormals; tanh's scale=1/32 undoes it).
  z = 32*(Wd @ dec + bias) bf16 replicated to 128 partitions.
- proj: 8 fp8 DoubleRow matmuls per tile into ONE [128,1024] f32 PSUM tile
  spanning 2 adjacent banks (ring of 3).  The z-add is then a single DVE
  tensor_add over all 1024 cols (measured 1.47us vs 1.63us for two).
- tanh runs on ACT over FOUR tiles at once ([128, 4096] in-place on a quad
  e-buffer): 0.98us/tile vs 1.36us single -- the per-instr overhead (~500ns)
  amortizes 4x.
- v-dot split by measured rates: DVE scalar_tensor_tensor w/accum fuses
  mult+reduce on cols [0:480] (1.27ns/col); Pool tensor_mul takes
  [480:1024] (2.0ns/col, idle engine) with ACT Copy+accum reducing that
  half (0.83ns/col + 0.5us fixed).  s = s0 + s1 via a tiny DVE add batched
  8 tiles wide, then one ACT exp per 8 tiles ([P,8]) -- exp cost drops
  from 336ns/tile to ~45ns/tile.
- ctx += p^T @ X: the two N=512 bf16 matmuls of a tile write partitions 0
  and 32 of the SAME PSUM bank = different PE column groups, so the PE
  overlaps them (tile_position auto-derives from the output base
  partition).  The batch's l-sum matmul rides partition 64.  ctx emits in
  2-tile groups to amortize fp8<->bf16 mode transitions.
- Softmax needs no max-subtraction (|scores| <= sum|v| <= 32).
"""
import os
import sys

sys.path.insert(0, "/opt/trn_rl_repo")

K_PJ2 = os.environ.get("K_PJ", "2") == "2"        # [128,1024] 2-bank pj tile
K_CTXCG = os.environ.get("K_CTX", "cg") == "cg"   # ctx col-group shared bank
K_TANHQ = os.environ.get("K_TANH", "q") == "q"    # quad tanh

from contextlib import ExitStack

import ml_dtypes
import numpy as np

import concourse.tile as tile
from concourse import bacc, mybir
from concourse.bass_utils import run_bass_kernel_spmd

F32 = mybir.dt.float32
BF16 = mybir.dt.bfloat16
FP8 = mybir.dt.float8e4
NP_FP8 = ml_dtypes.float8_e4m3
NP_BF16 = ml_dtypes.bfloat16
DR = mybir.MatmulPerfMode.DoubleRow

B, T, E, D = 64, 2048, 1024, 1024
CORES = 8
BL = B // CORES           # batches per core (8)
P = 128                   # partitions
TT = T // P               # t-tiles per batch (16)
ET = E // P               # e-blocks per row-tile (8)
QUAD = 4                  # t-tiles fetched per DMA instr / tanh'd per ACT instr
PREFETCH_QUADS = 4
WSCALE = 32.0             # fp8 subnormal-avoidance scale on We^T and z
ASPLIT = 480              # v-dot cols fused on DVE; rest Pool-mult + ACT-reduce

# pipeline lags (in t-tiles) behind emit_proj(k)
L_ZADD = 1                # DVE z-add of tile k-1 (quad-slice write)
# quad-tanh of tiles [k-4..k-1] fires when (k-1) completes a quad
L_VDOT = 5                # per-tile STT / Pool mult of tile k-5
L_RED = 6                 # ACT copy+accum of tile k-6
L_EXP = 7                 # s-add + exp of 4-group ending at k-7 (after its reduce)
L_CTX = 12                # ctx pair (k-12, k-11) on even kc
L_END = 14


def _build_kernel(bl=BL, t_tiles=TT):
    nc = bacc.Bacc(
        "TRN2",
        target_bir_lowering=False,
        debug=False,
        num_devices=CORES,
    )

    # [b, tl, i, e]: x16[b, tl, i, :] = enc[b, i*128+tl, :] in bf16
    x16 = nc.declare_dram_parameter("x16", [bl, P, t_tiles, E], BF16, isOutput=False)
    # [b, el, i, j, tl]: xt8[b, el, i, j, tl] = enc[b, i*128+tl, j*128+el] in fp8
    xt8 = nc.declare_dram_parameter("xt8", [bl, P, t_tiles, ET, P], FP8, isOutput=False)
    # [el, j, d]: wet8[el, j, d] = 32 * We[d, j*128+el] in fp8
    wet8 = nc.declare_dram_parameter("wet8", [P, ET, D], FP8, isOutput=False)
    # [k, b, d]: 32 * z[b, d] in bf16, replicated over k partitions
    zrep16 = nc.declare_dram_parameter("zrep16", [P, bl, D], BF16, isOutput=False)
    v16 = nc.declare_dram_parameter("v16", [P, D], BF16, isOutput=False)
    onesc = nc.declare_dram_parameter("onesc", [P, 1], BF16, isOutput=False)
    out = nc.declare_dram_parameter("ctx_out", [bl, E], F32, isOutput=True)

    n_quads_total = bl * t_tiles // QUAD

    with tile.TileContext(nc) as tc, ExitStack() as ctx:
        const = ctx.enter_context(tc.tile_pool(name="const", bufs=1))
        xq_pool = ctx.enter_context(tc.tile_pool(name="xq", bufs=6))
        xtq_pool = ctx.enter_context(tc.tile_pool(name="xtq", bufs=6))
        epool = ctx.enter_context(tc.tile_pool(name="e", bufs=5))
        small = ctx.enter_context(tc.tile_pool(name="small", bufs=2))

        n_proj_bufs = 3 if K_PJ2 else 5
        ps_proj = ctx.enter_context(
            tc.tile_pool(name="ps_proj", bufs=n_proj_bufs, space="PSUM"))
        ps_ctx = ctx.enter_context(tc.tile_pool(name="ps_ctx", bufs=2, space="PSUM"))
        if not (K_PJ2 and K_CTXCG):
            ps_misc = ctx.enter_context(
                tc.tile_pool(name="ps_misc", bufs=1, space="PSUM"))

        # ---- resident constants.  Ordered so proj(0)'s inputs land first.
        xq_tiles = {}
        xtq_tiles = {}

        def fetch_quad(q, skip_xq=False):
            b, qi = divmod(q, t_tiles // QUAD)
            if not skip_xq:
                xq = xq_pool.tile([P, QUAD, E], BF16, tag="xq")
                nc.sync.dma_start(xq[:], x16[b, :, QUAD * qi : QUAD * (qi + 1), :])
                xq_tiles[q] = xq
            xtq = xtq_pool.tile([P, QUAD, ET, P], FP8, tag="xtq")
            nc.sync.dma_start(xtq[:], xt8[b, :, QUAD * qi : QUAD * (qi + 1), :, :])
            xtq_tiles[q] = xtq

        fetch_quad(0, skip_xq=True)
        wet_t = []
        for pr in range(ET // 2):
            wt = const.tile([P, 2, D], FP8, name=f"wet{pr}")
            nc.sync.dma_start(wt[:], wet8[:, 2 * pr : 2 * pr + 2, :])
            wet_t.append(wt)
        z_t = [const.tile([P, D], BF16, name=f"z{b}") for b in range(bl)]
        nc.sync.dma_start(z_t[0][:], zrep16[:, 0])
        v_sb = const.tile([P, D], BF16)
        nc.sync.dma_start(v_sb[:], v16[:])
        xq0 = xq_pool.tile([P, QUAD, E], BF16, tag="xq")
        nc.sync.dma_start(xq0[:], x16[0, :, 0:QUAD, :])
        xq_tiles[0] = xq0
        onesc_sb = const.tile([P, 1], BF16)
        nc.sync.dma_start(onesc_sb[:], onesc[:])
        # scratch sinks (contents never read)
        junk = const.tile([P, ASPLIT], BF16, name="junk")
        prod = const.tile([P, 4, D - ASPLIT], BF16, name="prod")

        # ---- per-batch state ------------------------------------------------
        total = bl * t_tiles
        state = {}
        equads = {}   # quad index -> [128, QUAD, D] e-buffer

        def get_state(b):
            if b not in state:
                state[b] = dict(
                    s0=small.tile([P, t_tiles], F32, tag="s0", name=f"s0_{b}"),
                    s1=small.tile([P, t_tiles], F32, tag="s1", name=f"s1_{b}"),
                    s=small.tile([P, t_tiles], F32, tag="s", name=f"s_{b}"),
                    p_all=small.tile([P, t_tiles], BF16, tag="p", name=f"p_all_{b}"),
                    # one PSUM bank: ctx halves at partitions 0 / 32, l at 64
                    ctxb=(ps_ctx.tile([P, 512], F32, tag="ps_ctx", name=f"ctxb_{b}")
                          if K_CTXCG else None),
                    ctx0=(None if K_CTXCG else
                          ps_ctx.tile([1, 512], F32, tag="ps_ctx", name=f"ctx0_{b}")),
                    ctx1=(None if K_CTXCG else
                          ps_ctx.tile([1, 512], F32, tag="ps_ctx", name=f"ctx1_{b}")),
                    proj_ps=[None] * t_tiles,
                )
            return state[b]

        def emit_proj(b, i):
            # proj[t, d] = sum_e x[t, e] * 32*WeT[e, d], fp8 DoubleRow,
            # both 512-halves into one [128,1024] 2-bank PSUM tile
            st = get_state(b)
            k = b * t_tiles + i
            q, qi = divmod(k, QUAD)
            xtq = xtq_tiles[q]
            if K_PJ2:
                pj = ps_proj.tile([P, D], F32, tag="ps_proj", name=f"pj_{b}_{i}")
                pjs = [pj[:, 0:512], pj[:, 512:1024]]
                st["proj_ps"][i] = [pj]
            else:
                pj0 = ps_proj.tile([P, 512], F32, tag="ps_proj", name=f"pj0_{b}_{i}")
                pj1 = ps_proj.tile([P, 512], F32, tag="ps_proj", name=f"pj1_{b}_{i}")
                pjs = [pj0[:], pj1[:]]
                st["proj_ps"][i] = [pj0, pj1]
            for pr in range(ET // 2):
                lhs = xtq[:, qi, 2 * pr : 2 * pr + 2, :]
                for h in range(2):
                    nc.tensor.matmul(
                        pjs[h], lhs, wet_t[pr][:, :, 512 * h : 512 * (h + 1)],
                        start=(pr == 0), stop=(pr == ET // 2 - 1), perf_mode=DR,
                    )

        def emit_zadd(k):
            # e = (proj + 32z) bf16 into this quad's slice; one [128,1024]
            # DVE op spanning both PSUM banks
            b, i = divmod(k, t_tiles)
            st = get_state(b)
            q, qi = divmod(k, QUAD)
            if q not in equads:
                equads[q] = epool.tile([P, QUAD, D], BF16, tag="e", name=f"equad_{q}")
            pjl = st["proj_ps"][i]
            st["proj_ps"][i] = None
            if K_PJ2:
                nc.vector.tensor_add(equads[q][:, qi, :], pjl[0][:], z_t[b][:])
            else:
                for h in range(2):
                    sl = slice(512 * h, 512 * (h + 1))
                    nc.vector.tensor_add(
                        equads[q][:, qi, sl], pjl[h][:], z_t[b][:, sl]
                    )

        def emit_tanh_quad(q):
            # tanh over 4 tiles at once; ~0.98us/tile vs 1.36 single
            if K_TANHQ:
                nc.scalar.activation(
                    equads[q][:], equads[q][:], mybir.ActivationFunctionType.Tanh,
                    scale=1.0 / WSCALE,
                )
            else:
                for qi in range(QUAD):
                    nc.scalar.activation(
                        equads[q][:, qi, :], equads[q][:, qi, :],
                        mybir.ActivationFunctionType.Tanh, scale=1.0 / WSCALE,
                    )

        def emit_vdot(k):
            # cols [0:ASPLIT): DVE STT fused mult+accum -> s0
            # cols [ASPLIT:D): Pool mult -> prod (ACT reduces next step)
            b, i = divmod(k, t_tiles)
            st = get_state(b)
            q, qi = divmod(k, QUAD)
            e_sb = equads[q]
            nc.vector.scalar_tensor_tensor(
                out=junk[:],
                in0=e_sb[:, qi, 0:ASPLIT],
                scalar=1.0,
                in1=v_sb[:, 0:ASPLIT],
                op0=mybir.AluOpType.mult,
                op1=mybir.AluOpType.mult,
                accum_out=st["s0"][:, i : i + 1],
            )
            nc.gpsimd.tensor_mul(
                prod[:, k % 4, :], e_sb[:, qi, ASPLIT:D], v_sb[:, ASPLIT:D]
            )

        def emit_reduce(k):
            # ACT Copy+accum over Pool's product half -> s1
            b, i = divmod(k, t_tiles)
            st = get_state(b)
            nc.scalar.activation(
                prod[:, k % 4, :],
                prod[:, k % 4, :],
                mybir.ActivationFunctionType.Copy,
                accum_out=st["s1"][:, i : i + 1],
            )

        def emit_exp4(b, i0):
            # s = s0+s1 (DVE, [P,4]) then p = exp(s) (one ACT instr)
            st = get_state(b)
            sl = slice(i0, i0 + 4)
            nc.gpsimd.tensor_add(st["s"][:, sl], st["s0"][:, sl], st["s1"][:, sl])
            nc.scalar.activation(
                st["p_all"][:, sl], st["s"][:, sl],
                mybir.ActivationFunctionType.Exp,
            )

        def emit_ctx(b, i):
            # ctx_unnorm += p^T @ X; halves go to partitions 0 / 32 of one
            # PSUM bank = different PE column groups -> concurrent
            st = get_state(b)
            k = b * t_tiles + i
            q, qi = divmod(k, QUAD)
            xq = xq_tiles[q]
            p_col = st["p_all"][:, i : i + 1]
            for h in range(2):
                if K_CTXCG:
                    dst = st["ctxb"][32 * h : 32 * h + 1, :]
                else:
                    dst = st["ctx0" if h == 0 else "ctx1"][:]
                nc.tensor.matmul(
                    dst, p_col,
                    xq[:, qi, h * 512 : (h + 1) * 512],
                    start=(i == 0), stop=(i == t_tiles - 1),
                )

        def emit_batch_end(b):
            # l = sum_t exp(s_t); ctx = ctx_unnorm / l.  Partition sum via
            # DVE reduce + 1-col matmul into partition 64 of the ctx bank.
            st = state.pop(b)
            l_part = small.tile([P, 1], BF16, tag="lp")
            with nc.allow_low_precision(reason="l partials; err ~0.2%/sqrt(128)"):
                nc.vector.tensor_reduce(
                    l_part[:], st["p_all"][:],
                    axis=mybir.AxisListType.X, op=mybir.AluOpType.add,
                )
            if K_CTXCG:
                l_ps = st["ctxb"][64:65, 0:1]
                c0, c1 = st["ctxb"][0:1, :], st["ctxb"][32:33, :]
            else:
                lt = ps_misc.tile([1, 1], F32, tag="ps_misc", name=f"l_{b}")
                l_ps = lt[:]
                c0, c1 = st["ctx0"][:], st["ctx1"][:]
            nc.tensor.matmul(l_ps, l_part[:], onesc_sb[:])
            linv = small.tile([1, 1], F32, tag="linv")
            nc.vector.reciprocal(linv[:], l_ps)
            ctx_row = small.tile([1, E], F32, tag="ctxrow")
            nc.scalar.activation(
                ctx_row[:, 0:512], c0,
                mybir.ActivationFunctionType.Copy, scale=linv[:],
            )
            nc.scalar.activation(
                ctx_row[:, 512:E], c1,
                mybir.ActivationFunctionType.Copy, scale=linv[:],
            )
            nc.sync.dma_start(out[b : b + 1, :], ctx_row[:])

        # ---- main software pipeline over all (batch, t-tile) ----------------
        for k in range(total + L_END + 2):
            if k < total:
                emit_proj(*divmod(k, t_tiles))
            if k == 0:
                for q in range(1, PREFETCH_QUADS):
                    fetch_quad(q)
            if k % QUAD == 0:
                qf = k // QUAD + PREFETCH_QUADS
                if qf < n_quads_total:
                    fetch_quad(qf)
            if k % t_tiles == 8 and k // t_tiles + 1 < bl:
                b_next = k // t_tiles + 1
                nc.sync.dma_start(z_t[b_next][:], zrep16[:, b_next])
            kz = k - L_ZADD
            if 0 <= kz < total:
                emit_zadd(kz)
                if kz % QUAD == QUAD - 1:
                    emit_tanh_quad(kz // QUAD)
            kv = k - L_VDOT
            if 0 <= kv < total:
                emit_vdot(kv)
            kr = k - L_RED
            if 0 <= kr < total:
                emit_reduce(kr)
            ke = k - L_EXP
            if 0 <= ke < total and ke % 4 == 3:
                b_e, i_e = divmod(ke, t_tiles)
                emit_exp4(b_e, (i_e // 4) * 4)
            kc = k - L_CTX
            if kc >= 0 and kc % 2 == 0:
                for kk in (kc, kc + 1):
                    if 0 <= kk < total:
                        emit_ctx(*divmod(kk, t_tiles))
            kb = k - L_END
            if 0 <= kb < total and kb % t_tiles == t_tiles - 1:
                emit_batch_end(kb // t_tiles)
            # free the consumed e-quad once its last tile's reduce is done
            kq = k - L_RED - 1
            if kq >= 0 and kq % QUAD == QUAD - 1 and (kq // QUAD) in equads:
                equads.pop(kq // QUAD)

    nc.compile()
    return nc


def _prep_inputs(enc_out, dec_state, W_weight, W_bias, v_weight, bl=BL):
    """Host-side layout/dtype prep + per-core slicing."""
    enc_out = np.ascontiguousarray(enc_out, dtype=np.float32)
    dec_state = np.ascontiguousarray(dec_state, dtype=np.float32)
    W = np.asarray(W_weight, dtype=np.float32)

    # x16: [B, tl, i, e] bf16
    x16_h = np.ascontiguousarray(
        enc_out.reshape(B, TT, P, E).transpose(0, 2, 1, 3).astype(NP_BF16)
    )
    # xt8: [B, el, i, j, tl] fp8
    enc8 = enc_out.astype(NP_FP8)
    xt8_h = np.ascontiguousarray(
        enc8.reshape(B, TT, P, ET, P).transpose(0, 4, 1, 3, 2)
    )
    # wet8: [el, j, d], scaled by WSCALE to avoid e4m3 subnormals
    wet8_h = np.ascontiguousarray(
        (WSCALE * W[:, :E].T).astype(NP_FP8).reshape(ET, P, D).transpose(1, 0, 2)
    )
    # z = Wd @ dec + bias, scaled by WSCALE, bf16, replicated over k
    z_all = dec_state @ W[:, E:].T + np.asarray(W_bias, dtype=np.float32)  # [B, D]
    z16 = (WSCALE * z_all).astype(NP_BF16)
    v16_h = np.ascontiguousarray(
        np.broadcast_to(np.asarray(v_weight).astype(NP_BF16).reshape(1, D), (P, D))
    )
    onesc_h = np.ones((P, 1), dtype=NP_BF16)

    in_maps = []
    for c in range(CORES):
        sl = slice(c * bl, (c + 1) * bl)
        zrep_h = np.ascontiguousarray(np.broadcast_to(z16[None, sl], (P, bl, D)))
        in_maps.append(
            {
                "x16": x16_h[sl],
                "xt8": xt8_h[sl],
                "wet8": wet8_h,
                "zrep16": zrep_h,
                "v16": v16_h,
                "onesc": onesc_h,
            }
        )
    return in_maps


_NC_CACHE = {}


def _get_nc():
    if "nc" not in _NC_CACHE:
        _NC_CACHE["nc"] = _build_kernel()
    return _NC_CACHE["nc"]


def _run(inputs, trace=False, tmpdir=None):
    nc = _get_nc()
    in_maps = _prep_inputs(
        inputs["enc_out"],
        inputs["dec_state"],
        inputs["W_weight"],
        inputs["W_bias"],
        inputs["v_weight"],
    )
    res = run_bass_kernel_spmd(
        nc, in_maps, list(range(CORES)), trace=trace, tmpdir=tmpdir
    )
    out = np.concatenate(
        [np.asarray(res.results[c]["ctx_out"]) for c in range(CORES)], axis=0
    )
    return out.astype(np.float32, copy=False), res


def kernel(**inputs):
    out, _ = _run(inputs, trace=False)
    return out


if __name__ == "__main__":
    pass


# revision 16
# speedup vs baseline: 1.1890x; 1.0093x over previous
"""Trainium2 Bass kernel for additive (Bahdanau) attention.

  context[b] = sum_t softmax_t( v . tanh(We @ enc[b,t] + Wd @ dec[b] + bias) ) * enc[b,t]

Shapes (hardcoded): enc_out [64, 2048, 1024] f32, dec_state [64, 1024] f32,
W_weight [1024, 2048], W_bias [1024], v_weight [1, 1024].  Output [64, 1024].

Sharding: data-parallel over batch across 8 NeuronCores (8 batches/core).

Design v3 (measured-rate rebalance of every engine; v1 in kernel_v1_baseline):
- Host prep: enc cast twice -- bf16 [b, tl, i, e] (ctx moving stream) and
  fp8-e4m3 [b, el, i, j, tl] (proj stationary).  We^T scaled x32 in fp8
  [el, j, d] pairs (dodges e4m3 subnormals; tanh's scale=1/32 undoes it).
  z = 32*(Wd @ dec + bias) bf16 replicated to 128 partitions.
- proj: 8 fp8 DoubleRow matmuls per tile into ONE [128,1024] f32 PSUM tile
  spanning 2 adjacent banks (ring of 3).  The z-add is then a single DVE
  tensor_add over all 1024 cols (measured 1.47us vs 1.63us for two).
- tanh runs on ACT over FOUR tiles at once ([128, 4096] in-place on a quad
  e-buffer): 0.98us/tile vs 1.36us single -- the per-instr overhead (~500ns)
  amortizes 4x.
- v-dot split by measured rates: DVE scalar_tensor_tensor w/accum fuses
  mult+reduce on cols [0:480] (1.27ns/col); Pool tensor_mul takes
  [480:1024] (2.0ns/col, idle engine) with ACT Copy+accum reducing that
  half (0.83ns/col + 0.5us fixed).  s = s0 + s1 via a tiny DVE add batched
  8 tiles wide, then one ACT exp per 8 tiles ([P,8]) -- exp cost drops
  from 336ns/tile to ~45ns/tile.
- ctx += p^T @ X: the two N=512 bf16 matmuls of a tile write partitions 0
  and 32 of the SAME PSUM bank = different PE column groups, so the PE
  overlaps them (tile_position auto-derives from the output base
  partition).  The batch's l-sum matmul rides partition 64.  ctx emits in
  2-tile groups to amortize fp8<->bf16 mode transitions.
- Softmax needs no max-subtraction (|scores| <= sum|v| <= 32).
"""
import os
import sys

sys.path.insert(0, "/opt/trn_rl_repo")

K_PJ2 = os.environ.get("K_PJ", "2") == "2"        # [128,1024] 2-bank pj tile
K_CTXCG = os.environ.get("K_CTX", "cg") == "cg"   # ctx col-group shared bank
K_TANHQ = os.environ.get("K_TANH", "q") == "q"    # quad tanh

from contextlib import ExitStack

import ml_dtypes
import numpy as np

import concourse.tile as tile
from concourse import bacc, mybir
from concourse.bass_utils import run_bass_kernel_spmd

F32 = mybir.dt.float32
BF16 = mybir.dt.bfloat16
FP8 = mybir.dt.float8e4
NP_FP8 = ml_dtypes.float8_e4m3
NP_BF16 = ml_dtypes.bfloat16
DR = mybir.MatmulPerfMode.DoubleRow

B, T, E, D = 64, 2048, 1024, 1024
CORES = 8
BL = B // CORES           # batches per core (8)
P = 128                   # partitions
TT = T // P               # t-tiles per batch (16)
ET = E // P               # e-blocks per row-tile (8)
QUAD = 4                  # t-tiles fetched per DMA instr / tanh'd per ACT instr
PREFETCH_QUADS = 4
WSCALE = 32.0             # fp8 subnormal-avoidance scale on We^T and z
ASPLIT = 480              # v-dot cols fused on DVE; rest Pool-mult + ACT-reduce

# pipeline lags (in t-tiles) behind emit_proj(k)
L_ZADD = 1                # DVE z-add of tile k-1 (quad-slice write)
# quad-tanh of tiles [k-4..k-1] fires when (k-1) completes a quad
# quad-synchronized epilogue: when tile k-1 completes quad q, the whole
# quad's vdot fires at once, reduces next step, exp the step after.
L_RED = 2                 # quad reduce one step after quad vdot
L_EXP = 3                 # quad exp two steps after quad vdot
L_CTX = 7                 # ctx pair (k-7, k-6) on even kc
L_END = 9


def _build_kernel(bl=BL, t_tiles=TT):
    nc = bacc.Bacc(
        "TRN2",
        target_bir_lowering=False,
        debug=False,
        num_devices=CORES,
    )

    # [b, tl, i, e]: x16[b, tl, i, :] = enc[b, i*128+tl, :] in bf16
    x16 = nc.declare_dram_parameter("x16", [bl, P, t_tiles, E], BF16, isOutput=False)
    # [b, el, i, j, tl]: xt8[b, el, i, j, tl] = enc[b, i*128+tl, j*128+el] in fp8
    xt8 = nc.declare_dram_parameter("xt8", [bl, P, t_tiles, ET, P], FP8, isOutput=False)
    # [el, j, d]: wet8[el, j, d] = 32 * We[d, j*128+el] in fp8
    wet8 = nc.declare_dram_parameter("wet8", [P, ET, D], FP8, isOutput=False)
    # [k, b, d]: 32 * z[b, d] in bf16, replicated over k partitions
    zrep16 = nc.declare_dram_parameter("zrep16", [P, bl, D], BF16, isOutput=False)
    v16 = nc.declare_dram_parameter("v16", [P, D], BF16, isOutput=False)
    onesc = nc.declare_dram_parameter("onesc", [P, 1], BF16, isOutput=False)
    out = nc.declare_dram_parameter("ctx_out", [bl, E], F32, isOutput=True)

    n_quads_total = bl * t_tiles // QUAD

    with tile.TileContext(nc) as tc, ExitStack() as ctx:
        const = ctx.enter_context(tc.tile_pool(name="const", bufs=1))
        xq_pool = ctx.enter_context(tc.tile_pool(name="xq", bufs=6))
        xtq_pool = ctx.enter_context(tc.tile_pool(name="xtq", bufs=6))
        epool = ctx.enter_context(tc.tile_pool(name="e", bufs=5))
        small = ctx.enter_context(tc.tile_pool(name="small", bufs=2))

        n_proj_bufs = 3 if K_PJ2 else 5
        ps_proj = ctx.enter_context(
            tc.tile_pool(name="ps_proj", bufs=n_proj_bufs, space="PSUM"))
        ps_ctx = ctx.enter_context(tc.tile_pool(name="ps_ctx", bufs=2, space="PSUM"))
        if not (K_PJ2 and K_CTXCG):
            ps_misc = ctx.enter_context(
                tc.tile_pool(name="ps_misc", bufs=1, space="PSUM"))

        # ---- resident constants.  Ordered so proj(0)'s inputs land first.
        xq_tiles = {}
        xtq_tiles = {}

        def fetch_quad(q, skip_xq=False):
            b, qi = divmod(q, t_tiles // QUAD)
            if not skip_xq:
                xq = xq_pool.tile([P, QUAD, E], BF16, tag="xq")
                nc.sync.dma_start(xq[:], x16[b, :, QUAD * qi : QUAD * (qi + 1), :])
                xq_tiles[q] = xq
            xtq = xtq_pool.tile([P, QUAD, ET, P], FP8, tag="xtq")
            nc.sync.dma_start(xtq[:], xt8[b, :, QUAD * qi : QUAD * (qi + 1), :, :])
            xtq_tiles[q] = xtq

        fetch_quad(0, skip_xq=True)
        wet_t = []
        for pr in range(ET // 2):
            wt = const.tile([P, 2, D], FP8, name=f"wet{pr}")
            nc.sync.dma_start(wt[:], wet8[:, 2 * pr : 2 * pr + 2, :])
            wet_t.append(wt)
        z_t = [const.tile([P, D], BF16, name=f"z{b}") for b in range(bl)]
        nc.sync.dma_start(z_t[0][:], zrep16[:, 0])
        v_sb = const.tile([P, D], BF16)
        nc.sync.dma_start(v_sb[:], v16[:])
        xq0 = xq_pool.tile([P, QUAD, E], BF16, tag="xq")
        nc.sync.dma_start(xq0[:], x16[0, :, 0:QUAD, :])
        xq_tiles[0] = xq0
        onesc_sb = const.tile([P, 1], BF16)
        nc.sync.dma_start(onesc_sb[:], onesc[:])
        # scratch sinks (contents never read)
        junk = const.tile([P, ASPLIT], BF16, name="junk")
        prod = const.tile([P, 4, D - ASPLIT], BF16, name="prod")

        # ---- per-batch state ------------------------------------------------
        total = bl * t_tiles
        state = {}
        equads = {}   # quad index -> [128, QUAD, D] e-buffer

        def get_state(b):
            if b not in state:
                state[b] = dict(
                    s0=small.tile([P, t_tiles], F32, tag="s0", name=f"s0_{b}"),
                    s1=small.tile([P, t_tiles], F32, tag="s1", name=f"s1_{b}"),
                    s=small.tile([P, t_tiles], F32, tag="s", name=f"s_{b}"),
                    p_all=small.tile([P, t_tiles], BF16, tag="p", name=f"p_all_{b}"),
                    # one PSUM bank: ctx halves at partitions 0 / 32, l at 64
                    ctxb=(ps_ctx.tile([P, 512], F32, tag="ps_ctx", name=f"ctxb_{b}")
                          if K_CTXCG else None),
                    ctx0=(None if K_CTXCG else
                          ps_ctx.tile([1, 512], F32, tag="ps_ctx", name=f"ctx0_{b}")),
                    ctx1=(None if K_CTXCG else
                          ps_ctx.tile([1, 512], F32, tag="ps_ctx", name=f"ctx1_{b}")),
                    proj_ps=[None] * t_tiles,
                )
            return state[b]

        def emit_proj(b, i):
            # proj[t, d] = sum_e x[t, e] * 32*WeT[e, d], fp8 DoubleRow,
            # both 512-halves into one [128,1024] 2-bank PSUM tile
            st = get_state(b)
            k = b * t_tiles + i
            q, qi = divmod(k, QUAD)
            xtq = xtq_tiles[q]
            if K_PJ2:
                pj = ps_proj.tile([P, D], F32, tag="ps_proj", name=f"pj_{b}_{i}")
                pjs = [pj[:, 0:512], pj[:, 512:1024]]
                st["proj_ps"][i] = [pj]
            else:
                pj0 = ps_proj.tile([P, 512], F32, tag="ps_proj", name=f"pj0_{b}_{i}")
                pj1 = ps_proj.tile([P, 512], F32, tag="ps_proj", name=f"pj1_{b}_{i}")
                pjs = [pj0[:], pj1[:]]
                st["proj_ps"][i] = [pj0, pj1]
            for pr in range(ET // 2):
                lhs = xtq[:, qi, 2 * pr : 2 * pr + 2, :]
                for h in range(2):
                    nc.tensor.matmul(
                        pjs[h], lhs, wet_t[pr][:, :, 512 * h : 512 * (h + 1)],
                        start=(pr == 0), stop=(pr == ET // 2 - 1), perf_mode=DR,
                    )

        def emit_zadd(k):
            # e = (proj + 32z) bf16 into this quad's slice; one [128,1024]
            # DVE op spanning both PSUM banks
            b, i = divmod(k, t_tiles)
            st = get_state(b)
            q, qi = divmod(k, QUAD)
            if q not in equads:
                equads[q] = epool.tile([P, QUAD, D], BF16, tag="e", name=f"equad_{q}")
            pjl = st["proj_ps"][i]
            st["proj_ps"][i] = None
            if K_PJ2:
                nc.vector.tensor_add(equads[q][:, qi, :], pjl[0][:], z_t[b][:])
            else:
                for h in range(2):
                    sl = slice(512 * h, 512 * (h + 1))
                    nc.vector.tensor_add(
                        equads[q][:, qi, sl], pjl[h][:], z_t[b][:, sl]
                    )

        def emit_tanh_quad(q):
            # tanh over 4 tiles at once; ~0.98us/tile vs 1.36 single
            if K_TANHQ:
                nc.scalar.activation(
                    equads[q][:], equads[q][:], mybir.ActivationFunctionType.Tanh,
                    scale=1.0 / WSCALE,
                )
            else:
                for qi in range(QUAD):
                    nc.scalar.activation(
                        equads[q][:, qi, :], equads[q][:, qi, :],
                        mybir.ActivationFunctionType.Tanh, scale=1.0 / WSCALE,
                    )

        def emit_vdot(k):
            # cols [0:ASPLIT): DVE STT fused mult+accum -> s0
            # cols [ASPLIT:D): Pool mult -> prod (ACT reduces next step)
            b, i = divmod(k, t_tiles)
            st = get_state(b)
            q, qi = divmod(k, QUAD)
            e_sb = equads[q]
            nc.vector.scalar_tensor_tensor(
                out=junk[:],
                in0=e_sb[:, qi, 0:ASPLIT],
                scalar=1.0,
                in1=v_sb[:, 0:ASPLIT],
                op0=mybir.AluOpType.mult,
                op1=mybir.AluOpType.mult,
                accum_out=st["s0"][:, i : i + 1],
            )
            nc.gpsimd.tensor_mul(
                prod[:, k % 4, :], e_sb[:, qi, ASPLIT:D], v_sb[:, ASPLIT:D]
            )

        def emit_reduce(k):
            # ACT Copy+accum over Pool's product half -> s1
            b, i = divmod(k, t_tiles)
            st = get_state(b)
            nc.scalar.activation(
                prod[:, k % 4, :],
                prod[:, k % 4, :],
                mybir.ActivationFunctionType.Copy,
                accum_out=st["s1"][:, i : i + 1],
            )

        def emit_exp4(b, i0):
            # s = s0+s1 (DVE, [P,4]) then p = exp(s) (one ACT instr)
            st = get_state(b)
            sl = slice(i0, i0 + 4)
            nc.gpsimd.tensor_add(st["s"][:, sl], st["s0"][:, sl], st["s1"][:, sl])
            nc.scalar.activation(
                st["p_all"][:, sl], st["s"][:, sl],
                mybir.ActivationFunctionType.Exp,
            )

        def emit_ctx(b, i):
            # ctx_unnorm += p^T @ X; halves go to partitions 0 / 32 of one
            # PSUM bank = different PE column groups -> concurrent
            st = get_state(b)
            k = b * t_tiles + i
            q, qi = divmod(k, QUAD)
            xq = xq_tiles[q]
            p_col = st["p_all"][:, i : i + 1]
            for h in range(2):
                if K_CTXCG:
                    dst = st["ctxb"][32 * h : 32 * h + 1, :]
                else:
                    dst = st["ctx0" if h == 0 else "ctx1"][:]
                nc.tensor.matmul(
                    dst, p_col,
                    xq[:, qi, h * 512 : (h + 1) * 512],
                    start=(i == 0), stop=(i == t_tiles - 1),
                )

        def emit_batch_end(b):
            # l = sum_t exp(s_t); ctx = ctx_unnorm / l.  Partition sum via
            # DVE reduce + 1-col matmul into partition 64 of the ctx bank.
            st = state.pop(b)
            l_part = small.tile([P, 1], BF16, tag="lp")
            with nc.allow_low_precision(reason="l partials; err ~0.2%/sqrt(128)"):
                nc.vector.tensor_reduce(
                    l_part[:], st["p_all"][:],
                    axis=mybir.AxisListType.X, op=mybir.AluOpType.add,
                )
            if K_CTXCG:
                l_ps = st["ctxb"][64:65, 0:1]
                c0, c1 = st["ctxb"][0:1, :], st["ctxb"][32:33, :]
            else:
                lt = ps_misc.tile([1, 1], F32, tag="ps_misc", name=f"l_{b}")
                l_ps = lt[:]
                c0, c1 = st["ctx0"][:], st["ctx1"][:]
            nc.tensor.matmul(l_ps, l_part[:], onesc_sb[:])
            linv = small.tile([1, 1], F32, tag="linv")
            nc.vector.reciprocal(linv[:], l_ps)
            ctx_row = small.tile([1, E], F32, tag="ctxrow")
            nc.scalar.activation(
                ctx_row[:, 0:512], c0,
                mybir.ActivationFunctionType.Copy, scale=linv[:],
            )
            nc.scalar.activation(
                ctx_row[:, 512:E], c1,
                mybir.ActivationFunctionType.Copy, scale=linv[:],
            )
            nc.sync.dma_start(out[b : b + 1, :], ctx_row[:])

        # ---- main software pipeline over all (batch, t-tile) ----------------
        for k in range(total + L_END + 2):
            if k < total:
                emit_proj(*divmod(k, t_tiles))
            if k == 0:
                for q in range(1, PREFETCH_QUADS):
                    fetch_quad(q)
            if k % QUAD == 0:
                qf = k // QUAD + PREFETCH_QUADS
                if qf < n_quads_total:
                    fetch_quad(qf)
            if k % t_tiles == 8 and k // t_tiles + 1 < bl:
                b_next = k // t_tiles + 1
                nc.sync.dma_start(z_t[b_next][:], zrep16[:, b_next])
            kz = k - L_ZADD
            if 0 <= kz < total:
                emit_zadd(kz)
                if kz % QUAD == QUAD - 1:
                    qq = kz // QUAD
                    emit_tanh_quad(qq)
                    for kk in range(4 * qq, 4 * qq + 4):
                        emit_vdot(kk)
            kr = k - L_RED
            if 0 <= kr < total and kr % QUAD == QUAD - 1:
                for kk in range(kr - 3, kr + 1):
                    emit_reduce(kk)
            ke = k - L_EXP
            if 0 <= ke < total and ke % 4 == 3:
                b_e, i_e = divmod(ke, t_tiles)
                emit_exp4(b_e, (i_e // 4) * 4)
            kc = k - L_CTX
            if kc >= 0 and kc % 2 == 0:
                for kk in (kc, kc + 1):
                    if 0 <= kk < total:
                        emit_ctx(*divmod(kk, t_tiles))
            kb = k - L_END
            if 0 <= kb < total and kb % t_tiles == t_tiles - 1:
                emit_batch_end(kb // t_tiles)
            # free the consumed e-quad once its last tile's reduce is done
            kq = k - L_RED - 1
            if kq >= 0 and kq % QUAD == QUAD - 1 and (kq // QUAD) in equads:
                equads.pop(kq // QUAD)

    nc.compile()
    return nc


def _prep_inputs(enc_out, dec_state, W_weight, W_bias, v_weight, bl=BL):
    """Host-side layout/dtype prep + per-core slicing."""
    enc_out = np.ascontiguousarray(enc_out, dtype=np.float32)
    dec_state = np.ascontiguousarray(dec_state, dtype=np.float32)
    W = np.asarray(W_weight, dtype=np.float32)

    # x16: [B, tl, i, e] bf16
    x16_h = np.ascontiguousarray(
        enc_out.reshape(B, TT, P, E).transpose(0, 2, 1, 3).astype(NP_BF16)
    )
    # xt8: [B, el, i, j, tl] fp8
    enc8 = enc_out.astype(NP_FP8)
    xt8_h = np.ascontiguousarray(
        enc8.reshape(B, TT, P, ET, P).transpose(0, 4, 1, 3, 2)
    )
    # wet8: [el, j, d], scaled by WSCALE to avoid e4m3 subnormals
    wet8_h = np.ascontiguousarray(
        (WSCALE * W[:, :E].T).astype(NP_FP8).reshape(ET, P, D).transpose(1, 0, 2)
    )
    # z = Wd @ dec + bias, scaled by WSCALE, bf16, replicated over k
    z_all = dec_state @ W[:, E:].T + np.asarray(W_bias, dtype=np.float32)  # [B, D]
    z16 = (WSCALE * z_all).astype(NP_BF16)
    v16_h = np.ascontiguousarray(
        np.broadcast_to(np.asarray(v_weight).astype(NP_BF16).reshape(1, D), (P, D))
    )
    onesc_h = np.ones((P, 1), dtype=NP_BF16)

    in_maps = []
    for c in range(CORES):
        sl = slice(c * bl, (c + 1) * bl)
        zrep_h = np.ascontiguousarray(np.broadcast_to(z16[None, sl], (P, bl, D)))
        in_maps.append(
            {
                "x16": x16_h[sl],
                "xt8": xt8_h[sl],
                "wet8": wet8_h,
                "zrep16": zrep_h,
                "v16": v16_h,
                "onesc": onesc_h,
            }
        )
    return in_maps


_NC_CACHE = {}


def _get_nc():
    if "nc" not in _NC_CACHE:
        _NC_CACHE["nc"] = _build_kernel()
    return _NC_CACHE["nc"]


def _run(inputs, trace=False, tmpdir=None):
    nc = _get_nc()
    in_maps = _prep_inputs(
        inputs["enc_out"],
        inputs["dec_state"],
        inputs["W_weight"],
        inputs["W_bias"],
        inputs["v_weight"],
    )
    res = run_bass_kernel_spmd(
        nc, in_maps, list(range(CORES)), trace=trace, tmpdir=tmpdir
    )
    out = np.concatenate(
        [np.asarray(res.results[c]["ctx_out"]) for c in range(CORES)], axis=0
    )
    return out.astype(np.float32, copy=False), res


def kernel(**inputs):
    out, _ = _run(inputs, trace=False)
    return out


if __name__ == "__main__":
    pass
